# revision 1
# baseline (speedup 1.0000x reference)
"""Trainium2 Bass kernel for nn_DecoderBlock (B=4, S=1024, H=1024, 16 heads).

Sharding (8 cores, zero cross-core communication):
  core c -> batch b = c//2, half = c%2; own query rows are the interleaved
  256-row blocks {B0,B3} (half 0) / {B1,B2} (half 1) -- see own_rows().
  Row-parallel over the sequence for LN / projections / FFN; each core of a
  batch pair duplicates the full K/V projections (they depend only on the
  INPUTS x / key_and_value, never on the other core's partial results).

Device layout strategy:
  - fp16 operands for every matmul (1 cyc/row on PE), fp32 PSUM accumulate.
  - Attention in transposed-score space: scoresT[k, q] = kT.T @ qT per head,
    exp on ACT (scale=1/8 folded in), causal/padding mask as a 0/1 fp16
    MULTIPLY on the exp output (host-derived from the boolean masks; exact),
    softmax denominator via a ones-column appended to V (one extra PSUM row),
    normalized with reciprocal + gpsimd partition_broadcast.  ctxT feeds the
    output projection directly as lhsT - attention is never transposed.
  - Rows are host-permuted own-first so the uniform SPMD program slices "my
    queries" at column 0; softmax attention is key-order invariant and the
    mask tiles are built in permuted key order.  The interleaved block
    sharding makes the causally-clipped SA_SCHED (75% of the full rectangle,
    alternating full-width / upper-half-only key-chunk pairs) valid for BOTH
    cores of a pair with one uniform program; data masks cover the rest.
"""

import sys

sys.path.insert(0, "/opt/trn_rl_repo")

import time
from contextlib import ExitStack

import numpy as np

import concourse.bass as bass
import concourse.mybir as mybir
import concourse.tile as tile
from concourse import bacc
from concourse.masks import make_identity

F32 = mybir.dt.float32
F16 = mybir.dt.float16
AF = mybir.ActivationFunctionType
OP = mybir.AluOpType

B, S, H, NH, DK, FF = 4, 1024, 1024, 16, 64, 4096
P = 128
HT = H // P  # 8 feature tiles of the model dim
QN = 512  # own query rows per core
QC = QN // P  # 4 query chunks
RC = S // P  # 8 key/row chunks
FT = FF // P  # 32 ffn tiles
NEG = -50000.0  # fp16-safe -inf surrogate (exp(NEG/8) == 0 in fp32)
VW = 128  # padded per-head width of v_aug: [v(64) | ones | zeros(63)]
EPS = 1e-5

_CACHE: dict = {}
LAST_RUN_NS: int | None = None


def _ilv(k):
    """Row shuffle so that SBUF [P, k, n] loaded with "(p k) n -> p k n"
    (contiguous k*rowbytes per partition) holds orig row 128*c+p at
    (partition p, chunk c):  shuf[k*p + c] = orig[128*c + p]."""
    return (np.arange(k)[None, :] * 128 + np.arange(128)[:, None]).reshape(-1)


IDX8 = _ilv(8)
IDX4 = _ilv(4)
IDX32 = _ilv(32)


def _bcast_row_ap(dram_ap, parts=P):
    """DRAM [1, N] -> partition-broadcast AP [parts, N] (step-0 partition dim)."""
    return bass.AP(
        tensor=dram_ap.tensor, offset=dram_ap.offset, ap=[[0, parts], dram_ap.ap[1]]
    )


def _build(flags: frozenset, repeat: int = 1, ablate: frozenset = frozenset()):
    """Build + compile the single SPMD program. `flags` toggles optional ops."""
    use_ca_mask = "ca_mask" in flags
    use_sa_full = "sa_full" in flags
    ln_g = {i: f"ln{i}_g" in flags for i in (1, 2, 3)}
    ln_b = {i: f"ln{i}_b" in flags for i in (1, 2, 3)}
    use_b2 = "b2" in flags

    nc = bacc.Bacc("TRN2", target_bir_lowering=False, debug=False, num_devices=8)

    D = {}

    def din(name, shape, dt):
        D[name] = nc.dram_tensor(name, shape, dt, kind="ExternalInput").ap()

    din("x_own", [QN, H], F16)
    din("x_rm", [S, H], F16)  # permuted rows (own first)
    din("kv_rm", [S, H], F16)
    din("sa_cb", [P, RC, QN], F16)  # additive causal bias, permuted key order
    if use_ca_mask:
        din("ca_cb", [P, RC, QN], F16)
    for w in ("sa_wq", "sa_wk", "sa_wv", "sa_wo", "ca_wq", "ca_wk", "ca_wv", "ca_wo"):
        din(w, [H, H], F16)
    din("fc_w1", [H, FF], F16)
    din("fc_w2", [FF, H], F16)
    din("fc_b1f", [P, FT], F32)  # feature-major b1
    if use_b2:
        din("fc_b2", [1, H], F32)
    for i in (1, 2, 3):
        if ln_g[i]:
            din(f"ln{i}_g", [1, H], F32)
        if ln_b[i]:
            din(f"ln{i}_b", [1, H], F32)
    out_d = nc.dram_tensor("out", [QN, H], F32, kind="ExternalOutput").ap()

    with tile.TileContext(nc) as tc, ExitStack() as top:
        sing = top.enter_context(tc.tile_pool(name="sing", bufs=1))
        ident = sing.tile([P, P], F16)
        make_identity(nc, ident)
        eps_t = sing.tile([P, 1], F32)
        nc.vector.memset(eps_t, EPS)
        ones_c = sing.tile([P, 1], F32)
        nc.vector.memset(ones_c, 1.0)

        # broadcast [1,H] params across partitions via step-0 DMA
        gb_tiles = {}
        for i in (1, 2, 3):
            for kind, on in (("g", ln_g[i]), ("b", ln_b[i])):
                if on:
                    t = sing.tile([P, H], F32)
                    nc.sync.dma_start(out=t, in_=_bcast_row_ap(D[f"ln{i}_{kind}"]))
                    gb_tiles[(i, kind)] = t
        b1f = sing.tile([P, FT], F32)
        nc.sync.dma_start(out=b1f, in_=D["fc_b1f"])
        b2_t = None
        if use_b2:
            b2_t = sing.tile([P, H], F32)
            nc.sync.dma_start(out=b2_t, in_=_bcast_row_ap(D["fc_b2"]))

        small = top.enter_context(tc.tile_pool(name="small", bufs=4))
        norms = top.enter_context(tc.tile_pool(name="norms", bufs=2))
        scratch = top.enter_context(tc.tile_pool(name="scratch", bufs=2))
        lnzp = top.enter_context(tc.tile_pool(name="lnzp", bufs=1))
        kvsrc = top.enter_context(tc.tile_pool(name="kvsrc", bufs=1))

        def emit_once():
         xown16 = sing.tile([P, QC, H], F16, tag="xown")
         nc.sync.dma_start(
             out=xown16, in_=D["x_own"].rearrange("(p q) n -> p q n", p=P)
         )
         sacb = sing.tile([P, RC, QN], F16, tag="sacb")
         nc.sync.dma_start(out=sacb, in_=D["sa_cb"])
         cacb = None
         if use_ca_mask:
             cacb = sing.tile([P, RC, QN], F16, tag="cacb")
             nc.sync.dma_start(out=cacb, in_=D["ca_cb"])
         z = sing.tile([P, QC, H], F16, tag="z")
         z2 = sing.tile([P, QC, H], F16, tag="z2")

         mm_stack = ExitStack()
         ps_mm = mm_stack.enter_context(
             tc.tile_pool(name="ps_mm", bufs=2, space="PSUM")
         )

         def layernorm_T(row_ap_fn, n_rc, lnT, ln_idx, ps_t, tag):
            """Row-major LN stats+apply, then PE-transpose into feature-major lnT.

            row_ap_fn(rc) -> SBUF AP [P, H] holding rows rc*128..+128.
            lnT: [P, HT, n_rc*P] fp16 tile (feat-in-tile, feat-tile, row).
            """
            for rc in range(n_rc):
                row = row_ap_fn(rc)
                st = small.tile([P, 2, 6], F32, tag="st")
                nc.vector.bn_stats(out=st[:, 0], in_=row[:, 0:512])
                nc.vector.bn_stats(out=st[:, 1], in_=row[:, 512:H])
                mv = small.tile([P, 2], F32, tag="mv")
                nc.vector.bn_aggr(out=mv, in_=st)
                rstd = small.tile([P, 1], F32, tag="rstd")
                nc.scalar.activation(
                    out=rstd, in_=mv[:, 1:2], func=AF.Sqrt, bias=eps_t, scale=1.0
                )
                nc.vector.reciprocal(out=rstd, in_=rstd)
                lnr = scratch.tile([P, H], F16, tag="lnr")
                nc.vector.tensor_scalar(
                    out=lnr,
                    in0=row,
                    scalar1=mv[:, 0:1],
                    scalar2=rstd,
                    op0=OP.subtract,
                    op1=OP.mult,
                )
                if (ln_idx, "g") in gb_tiles:
                    nc.vector.tensor_mul(out=lnr, in0=lnr, in1=gb_tiles[(ln_idx, "g")])
                if (ln_idx, "b") in gb_tiles:
                    nc.vector.tensor_add(out=lnr, in0=lnr, in1=gb_tiles[(ln_idx, "b")])
                for f in range(HT):
                    pt = ps_t.tile([P, P], F16, tag="pt")
                    nc.tensor.transpose(pt, lnr[:, f * P : (f + 1) * P], ident)
                    nc.vector.tensor_copy(
                        out=lnT[:, f, rc * P : (rc + 1) * P], in_=pt
                    )

         _w8_cache = {}

         def load_w8(dram, pool):
            if "nowdma" in ablate:
                if "w8" not in _w8_cache:
                    t = pool.tile([P, HT, H], F16, tag="w8")
                    nc.sync.dma_start(
                        out=t, in_=dram.rearrange("(p k) n -> p k n", p=P)
                    )
                    _w8_cache["w8"] = t
                return _w8_cache["w8"]
            t = pool.tile([P, HT, H], F16, tag="w8")
            nc.sync.dma_start(out=t, in_=dram.rearrange("(p k) n -> p k n", p=P))
            return t

         FULL_SCHED = ((512, 0),) * RC
         # uniform causally-clipped SA schedule (see _prep_core block layout):
         # kc pairs alternate full-width and upper-half-only (q cols 256..511)
         SA_SCHED = (
             (512, 0), (512, 0), (256, 256), (256, 256),
             (512, 0), (512, 0), (256, 256), (256, 256),
         )

         def attention(qT, kT, v_aug, ctxT, cb, ps_s, ps_av, exp_pool, sched):
            for h in range(NH):
                f, r0 = h // 2, (h % 2) * 64
                pm_av = (
                    None
                    if "noav" in ablate
                    else ps_av.tile([P, QN], F32, tag="av")
                )
                for g in range(4):
                    n, off = sched[2 * g]  # both kc of a pair share (n, off)
                    et = exp_pool.tile([P, 2, QN], F16, tag="et")
                    if "noscores" in ablate:
                        nc.vector.memset(et, 0.01)
                    else:
                        pm_s = ps_s.tile([P, 2, QN], F32, tag="s")
                        for j in range(2):
                            kc = g * 2 + j
                            # K=128 contraction: other head's rows in qT are 0
                            nc.tensor.matmul(
                                pm_s[:, j, 0:n],
                                lhsT=kT[:, f, kc * P : (kc + 1) * P],
                                rhs=qT[:, h, off : off + n],
                                start=True,
                                stop=True,
                            )
                        if "noexp" in ablate:
                            nc.vector.tensor_copy(
                                out=et[:, :, 0:n], in_=pm_s[:, :, 0:n]
                            )
                        else:
                            nc.scalar.activation(
                                out=et[:, :, 0:n],
                                in_=pm_s[:, :, 0:n],
                                func=AF.Exp,
                                scale=0.125,
                            )
                        if cb is not None and "nobias" not in ablate:
                            # exp(s/8)*m == masked softmax numerator (m in {0,1})
                            nc.vector.tensor_mul(
                                out=et[:, :, 0:n],
                                in0=et[:, :, 0:n],
                                in1=cb[:, 2 * g : 2 * g + 2, off : off + n],
                            )
                    if "noav" not in ablate:
                        for j in range(2):
                            kc = g * 2 + j
                            # lhsT = [v(64) | ones | zeros(63)]: psum row 64
                            # accumulates sumexp; rows 65+ are never read.
                            nc.tensor.matmul(
                                pm_av[:, off : off + n],
                                lhsT=v_aug[:, kc, h, :],
                                rhs=et[:, j, 0:n],
                                start=(kc == 0),
                                stop=(kc == RC - 1),
                            )
                if "noav" in ablate:
                    nc.vector.memset(ctxT[r0 : r0 + 64, f, :], 0.25)
                elif "nonorm" in ablate:
                    nc.vector.tensor_copy(
                        out=ctxT[r0 : r0 + 64, f, :], in_=pm_av[0:64, :]
                    )
                else:
                    recip = norms.tile([1, QN], F16, tag="rec")
                    with nc.allow_low_precision(
                        reason="1/sumexp in fp16 is within output tolerance"
                    ):
                        nc.vector.reciprocal(out=recip, in_=pm_av[64:65, :])
                    rb = norms.tile([64, QN], F16, tag="rb")
                    nc.gpsimd.partition_broadcast(out_ap=rb, in_ap=recip)
                    nc.vector.tensor_mul(
                        out=ctxT[r0 : r0 + 64, f, :], in0=pm_av[0:64, :], in1=rb
                    )

         def proj_heads_qpad(qT_pad, w_sb, lnT):
            # qT_pad[:, h, :]: head h q-dims at rows (h%2)*64..+64 (matching its
            # row range inside the packed kT tile f=h//2), other 64 rows zero.
            nc.vector.memset(qT_pad, 0.0)
            for f in range(HT):
                pm = ps_mm.tile([P, 512], F32, tag="proj")
                for kc in range(HT):
                    nc.tensor.matmul(
                        pm,
                        lhsT=w_sb[:, kc, f * P : (f + 1) * P],
                        rhs=lnT[:, kc, 0:QN],
                        start=(kc == 0),
                        stop=(kc == HT - 1),
                    )
                nc.vector.tensor_copy(out=qT_pad[0:64, 2 * f, :], in_=pm[0:64, :])
                nc.vector.tensor_copy(
                    out=qT_pad[64:128, 2 * f + 1, :], in_=pm[64:128, :]
                )

         def proj_to_featmajor(outT, w_sb, lnT, n_cols):
            # outT[:, f, c*512:+512] = sum_kc w[kc,f]^T @ lnT[kc, cols]
            for f in range(HT):
                for c in range(n_cols // 512):
                    pm = ps_mm.tile([P, 512], F32, tag="proj")
                    for kc in range(HT):
                        nc.tensor.matmul(
                            pm,
                            lhsT=w_sb[:, kc, f * P : (f + 1) * P],
                            rhs=lnT[:, kc, c * 512 : (c + 1) * 512],
                            start=(kc == 0),
                            stop=(kc == HT - 1),
                        )
                    nc.vector.tensor_copy(
                        out=outT[:, f, c * 512 : (c + 1) * 512], in_=pm
                    )

         def make_v_aug(v_aug, w_sb, lnT):
            nc.vector.memset(v_aug[:, :, :, 64:VW], 0.0)
            nc.vector.tensor_copy(
                out=v_aug[:, :, :, 64:65], in_=ones_c.to_broadcast([P, RC, NH, 1])
            )
            for kc in range(RC):
                for vc in range(2):
                    pm = ps_mm.tile([P, 512], F32, tag="proj")
                    for hc in range(HT):
                        nc.tensor.matmul(
                            pm,
                            lhsT=lnT[:, hc, kc * P : (kc + 1) * P],
                            rhs=w_sb[:, hc, vc * 512 : (vc + 1) * 512],
                            start=(hc == 0),
                            stop=(hc == HT - 1),
                        )
                    nc.vector.tensor_copy(
                        out=v_aug[:, kc, vc * 8 : (vc + 1) * 8, 0:64],
                        in_=pm.rearrange("p (h d) -> p h d", h=8),
                    )

         def wo_residual(ctxT, w_sb, base, out_rows):
            # out_rows[:, qc, :] = base[:, qc, :] + ctx @ wo
            for qc in range(QC):
                for ncol in range(2):
                    pm = ps_mm.tile([P, 512], F32, tag="proj")
                    for hd in range(HT):
                        nc.tensor.matmul(
                            pm,
                            lhsT=ctxT[:, hd, qc * P : (qc + 1) * P],
                            rhs=w_sb[:, hd, ncol * 512 : (ncol + 1) * 512],
                            start=(hd == 0),
                            stop=(hd == HT - 1),
                        )
                    sl = slice(ncol * 512, (ncol + 1) * 512)
                    nc.vector.tensor_tensor(
                        out=out_rows[:, qc, sl], in0=pm, in1=base[:, qc, sl], op=OP.add
                    )

         lnz = lnzp.tile([P, HT, QN], F16, tag="lnzT")

         with tc.tile_pool(name="attn_acts", bufs=1) as acts, tc.tile_pool(
            name="wpool", bufs=2
         ) as wpool:
            # ---------------- P0: LN1(x) -> ln1T ----------------
            x_sb = acts.tile([P, RC, H], F16, tag="kT")
            nc.sync.dma_start(
                out=x_sb, in_=D["x_rm"].rearrange("(p k) n -> p k n", p=P)
            )
            ln1T = acts.tile([P, HT, S], F16, tag="lnT")
            with tc.tile_pool(name="ps_t1", bufs=2, space="PSUM") as ps_t:
                layernorm_T(lambda rc: x_sb[:, rc, :], RC, ln1T, 1, ps_t, "l1")

            # ---------------- P1: SA projections ----------------
            qT = acts.tile([P, NH, QN], F16, tag="qT")
            kT = acts.tile([P, HT, S], F16, tag="kT")
            v_aug = acts.tile([P, RC, NH, VW], F16, tag="vaug")
            wq = load_w8(D["sa_wq"], wpool)
            proj_heads_qpad(qT, wq, ln1T)
            wk = load_w8(D["sa_wk"], wpool)
            proj_to_featmajor(kT, wk, ln1T, S)
            wv = load_w8(D["sa_wv"], wpool)
            make_v_aug(v_aug, wv, ln1T)

            # ---------------- SA attention ----------------
            ctxT = acts.tile([P, HT, QN], F16, tag="ctxT")
            if "noattn" in ablate:
                nc.vector.memset(ctxT, 0.25)
            else:
                with (
                    tc.tile_pool(name="ps_s1", bufs=2, space="PSUM") as ps_s,
                    tc.tile_pool(name="ps_av1", bufs=2, space="PSUM") as ps_av,
                    tc.tile_pool(name="exp1", bufs=3) as exp_pool,
                ):
                    attention(
                     qT, kT, v_aug, ctxT, sacb, ps_s, ps_av, exp_pool,
                     FULL_SCHED if use_sa_full else SA_SCHED,
                 )

            # ---------------- SA wo + residual -> z ----------------
            wo = load_w8(D["sa_wo"], wpool)
            wo_residual(ctxT, wo, xown16, z)

            # ---------------- P2: cross attention ----------------
            kv_sb = kvsrc.tile([P, RC, H], F16, tag="kvrows")
            nc.sync.dma_start(
                out=kv_sb, in_=D["kv_rm"].rearrange("(p k) n -> p k n", p=P)
            )
            ln2kvT = acts.tile([P, HT, S], F16, tag="lnT")  # reuses ln1T slot
            with tc.tile_pool(name="ps_t2", bufs=2, space="PSUM") as ps_t:
                layernorm_T(lambda rc: kv_sb[:, rc, :], RC, ln2kvT, 2, ps_t, "l2kv")
                layernorm_T(lambda rc: z[:, rc, :], QC, lnz, 2, ps_t, "l2z")

            qT2 = acts.tile([P, NH, QN], F16, tag="qT")
            kT2 = acts.tile([P, HT, S], F16, tag="kT")
            v_aug2 = acts.tile([P, RC, NH, VW], F16, tag="vaug")
            cwq = load_w8(D["ca_wq"], wpool)
            proj_heads_qpad(qT2, cwq, lnz)
            cwk = load_w8(D["ca_wk"], wpool)
            proj_to_featmajor(kT2, cwk, ln2kvT, S)
            cwv = load_w8(D["ca_wv"], wpool)
            make_v_aug(v_aug2, cwv, ln2kvT)

            ctxT2 = acts.tile([P, HT, QN], F16, tag="ctxT")
            if "noattn" in ablate:
                nc.vector.memset(ctxT2, 0.25)
            else:
                with (
                    tc.tile_pool(name="ps_s2", bufs=2, space="PSUM") as ps_s,
                    tc.tile_pool(name="ps_av2", bufs=2, space="PSUM") as ps_av,
                    tc.tile_pool(name="exp2", bufs=3) as exp_pool,
                ):
                    attention(qT2, kT2, v_aug2, ctxT2, cacb, ps_s, ps_av, exp_pool, FULL_SCHED)

            cwo = load_w8(D["ca_wo"], wpool)
            wo_residual(ctxT2, cwo, z, z2)

         # ---------------- P3: FFN ----------------
         with tc.tile_pool(name="ps_t3", bufs=2, space="PSUM") as ps_t:
            layernorm_T(lambda rc: z2[:, rc, :], QC, lnz, 3, ps_t, "l3")

         with (
             tc.tile_pool(name="hTpool", bufs=1) as hTpool,
             tc.tile_pool(name="w2pool", bufs=3) as w2pool,
         ):
            hT = hTpool.tile([P, FT, QN], F16, tag="hT")
            w2_ap = D["fc_w2"].rearrange("(p k) n -> p k n", p=P)
            with tc.tile_pool(name="w1pool", bufs=1) as w1pool:
                # two halves so hT matmuls start after the first 4MB lands
                w1a = w1pool.tile([P, 4, FF], F16, tag="w1a")
                nc.sync.dma_start(
                    out=w1a,
                    in_=D["fc_w1"].rearrange("(p k) n -> p k n", p=P)[:, 0:4, :],
                )
                w1b = w1pool.tile([P, 4, FF], F16, tag="w1b")
                nc.sync.dma_start(
                    out=w1b,
                    in_=D["fc_w1"].rearrange("(p k) n -> p k n", p=P)[:, 4:8, :],
                )
                for ft in range(FT):
                    pm = ps_mm.tile([P, 512], F32, tag="proj")
                    for kc in range(HT):
                        w1t = w1a if kc < 4 else w1b
                        nc.tensor.matmul(
                            pm,
                            lhsT=w1t[:, kc % 4, ft * P : (ft + 1) * P],
                            rhs=lnz[:, kc, :],
                            start=(kc == 0),
                            stop=(kc == HT - 1),
                        )
                    nc.scalar.activation(
                        out=hT[:, ft, :],
                        in_=pm,
                        func=AF.Relu,
                        bias=b1f[:, ft : ft + 1],
                    )

            mm_stack.close()  # free ps_mm banks for ps_big
            out_rows = sing.tile([P, QC, H], F32, tag="z")  # reuses z slot
            with tc.tile_pool(name="ps_big", bufs=1, space="PSUM") as ps_big:
                pm8 = ps_big.tile([P, 8, 512], F32)
                w2t = None
                for kc in range(FT):
                    if kc % 4 == 0:
                        w2t = w2pool.tile([P, 4, H], F16, tag="w2s")
                        nc.sync.dma_start(
                            out=w2t, in_=w2_ap[:, kc : kc + 4, :]
                        )
                    for qc in range(QC):
                        for ncol in range(2):
                            nc.tensor.matmul(
                                pm8[:, qc * 2 + ncol, :],
                                lhsT=hT[:, kc, qc * P : (qc + 1) * P],
                                rhs=w2t[:, kc % 4, ncol * 512 : (ncol + 1) * 512],
                                start=(kc == 0),
                                stop=(kc == FT - 1),
                            )
                for qc in range(QC):
                    for ncol in range(2):
                        sl = slice(ncol * 512, (ncol + 1) * 512)
                        nc.vector.tensor_tensor(
                            out=out_rows[:, qc, sl],
                            in0=pm8[:, qc * 2 + ncol, :],
                            in1=z2[:, qc, sl],
                            op=OP.add,
                        )
                        if b2_t is not None:
                            nc.vector.tensor_add(
                                out=out_rows[:, qc, sl],
                                in0=out_rows[:, qc, sl],
                                in1=b2_t[:, sl],
                            )
         nc.sync.dma_start(
             out=out_d.rearrange("(q p) n -> p q n", p=P), in_=out_rows
         )

        for _ in range(repeat):
            emit_once()

    nc.compile()
    return nc


def own_rows(half):
    """Query rows of a core: blocks {B0,B3} / {B1,B2} of 256 rows each.
    This interleaving is what makes the uniform SA_SCHED causally valid
    for both cores of a batch pair."""
    if half == 0:
        return np.concatenate([np.arange(0, 256), np.arange(768, 1024)])
    return np.arange(256, 768)


def _prep_core(c, x, kv, future_mask, mask, use_ca_mask):
    b, half = c // 2, c % 2
    own = own_rows(half)
    if half == 0:
        rest = np.concatenate([np.arange(256, 512), np.arange(512, 768)])
    else:
        rest = np.concatenate([np.arange(0, 256), np.arange(768, 1024)])
    perm = np.concatenate([own, rest])
    m = {}
    m["x_own"] = np.ascontiguousarray(x[b, own][IDX4]).astype(np.float16)
    m["x_rm"] = np.ascontiguousarray(x[b][perm][IDX8]).astype(np.float16)
    m["kv_rm"] = np.ascontiguousarray(kv[b][IDX8]).astype(np.float16)
    # sa_cb[p, kc, q] = 0 where future_mask[b, own_q, perm_key] else 1 (key=kc*128+p)
    fm = future_mask[b, own][:, perm]  # [QN, S] bool
    cb = np.where(fm.T, np.float16(0.0), np.float16(1.0))  # [S, QN]
    m["sa_cb"] = np.ascontiguousarray(cb.reshape(RC, P, QN).transpose(1, 0, 2))
    if use_ca_mask:
        cm = mask[b, own]  # [QN, S]
        ccb = np.where(cm.T, np.float16(0.0), np.float16(1.0))
        m["ca_cb"] = np.ascontiguousarray(ccb.reshape(RC, P, QN).transpose(1, 0, 2))
    return m


def _prep_shared(inp):
    shared = {}
    for w in ("sa_wq", "sa_wk", "sa_wv", "sa_wo", "ca_wq", "ca_wk", "ca_wv", "ca_wo"):
        shared[w] = np.ascontiguousarray(np.asarray(inp[w])[IDX8]).astype(np.float16)
    shared["fc_w1"] = np.ascontiguousarray(
        np.asarray(inp["fc_w1"])[IDX8]
    ).astype(np.float16)
    shared["fc_w2"] = np.ascontiguousarray(
        np.asarray(inp["fc_w2"])[IDX32]
    ).astype(np.float16)
    shared["fc_b1f"] = np.ascontiguousarray(
        np.asarray(inp["fc_b1"]).reshape(FT, P).T
    ).astype(np.float32)
    return shared


def kernel(**inputs) -> np.ndarray:
    global LAST_RUN_NS
    inp = {k: np.asarray(v) for k, v in inputs.items()}
    x, kv = inp["x"], inp["key_and_value"]
    mask, future_mask = inp["mask"], inp["future_mask"]

    flags = set()
    if mask.any():
        flags.add("ca_mask")
    # The clipped SA_SCHED structurally skips regions that a standard causal
    # mask guarantees are masked.  Only safe if future_mask IS causal triu;
    # otherwise fall back to the full-rectangle schedule (mask data covers it).
    tri = np.triu(np.ones((S, S), dtype=bool), 1)
    if not all(np.array_equal(future_mask[b], tri) for b in range(B)):
        flags.add("sa_full")
    for i in (1, 2, 3):
        if not np.all(inp[f"ln{i}_g"] == 1.0):
            flags.add(f"ln{i}_g")
        if np.any(inp[f"ln{i}_b"] != 0.0):
            flags.add(f"ln{i}_b")
    if np.any(inp["fc_b2"] != 0.0):
        flags.add("b2")
    flags = frozenset(flags)

    if flags not in _CACHE:
        _CACHE[flags] = _build(flags)
    nc = _CACHE[flags]

    shared = _prep_shared(inp)
    if "b2" in flags:
        shared["fc_b2"] = inp["fc_b2"].reshape(1, H).astype(np.float32)
    for i in (1, 2, 3):
        if f"ln{i}_g" in flags:
            shared[f"ln{i}_g"] = inp[f"ln{i}_g"].reshape(1, H).astype(np.float32)
        if f"ln{i}_b" in flags:
            shared[f"ln{i}_b"] = inp[f"ln{i}_b"].reshape(1, H).astype(np.float32)

    in_maps = []
    for c in range(8):
        m = _prep_core(c, x, kv, future_mask, mask, "ca_mask" in flags)
        m.update(shared)
        in_maps.append(m)

    from concourse import bass_utils

    t0 = time.perf_counter_ns()
    res = bass_utils.run_bass_kernel_spmd(
        nc, in_maps, core_ids=list(range(8)), trace=False
    )
    LAST_RUN_NS = time.perf_counter_ns() - t0

    out = np.empty((B, S, H), np.float32)
    for c in range(8):
        b, half = c // 2, c % 2
        out[b, own_rows(half)] = res.results[c]["out"]
    return out



# revision 12
# speedup vs baseline: 7.9892x; 7.9892x over previous
"""Trainium2 Bass kernel for nn_DecoderBlock (B=4, S=1024, H=1024, 16 heads).

Sharding (8 cores, zero cross-core communication):
  core c -> batch b = c//2, half = c%2; own query rows are the interleaved
  256-row blocks {B0,B3} (half 0) / {B1,B2} (half 1) -- see own_rows().
  Row-parallel over the sequence for LN / projections / FFN; each core of a
  batch pair duplicates the full K/V projections (they depend only on the
  INPUTS x / key_and_value, never on the other core's partial results).

Device layout strategy:
  - fp16 operands for every matmul (1 cyc/row on PE), fp32 PSUM accumulate.
  - Attention in transposed-score space: scoresT[k, q] = kT.T @ qT per head,
    exp on ACT (scale=1/8 folded in), causal/padding mask as a 0/1 fp16
    MULTIPLY on the exp output (host-derived from the boolean masks; exact),
    softmax denominator via a ones-column appended to V (one extra PSUM row),
    normalized with reciprocal + gpsimd partition_broadcast.  ctxT feeds the
    output projection directly as lhsT - attention is never transposed.
  - Rows are host-permuted own-first so the uniform SPMD program slices "my
    queries" at column 0; softmax attention is key-order invariant and the
    mask tiles are built in permuted key order.  The interleaved block
    sharding makes the causally-clipped SA_SCHED (75% of the full rectangle,
    alternating full-width / upper-half-only key-chunk pairs) valid for BOTH
    cores of a pair with one uniform program; data masks cover the rest.
"""

import sys

sys.path.insert(0, "/opt/trn_rl_repo")

import time
from contextlib import ExitStack

import numpy as np

import concourse.bass as bass
import concourse.mybir as mybir
import concourse.tile as tile
from concourse import bacc
from concourse.masks import make_identity

F32 = mybir.dt.float32
F16 = mybir.dt.float16
AF = mybir.ActivationFunctionType
OP = mybir.AluOpType

B, S, H, NH, DK, FF = 4, 1024, 1024, 16, 64, 4096
P = 128
HT = H // P  # 8 feature tiles of the model dim
QN = 512  # own query rows per core
QC = QN // P  # 4 query chunks
RC = S // P  # 8 key/row chunks
FT = FF // P  # 32 ffn tiles
NEG = -50000.0  # fp16-safe -inf surrogate (exp(NEG/8) == 0 in fp32)
VW = 72  # per-head width of v_aug: [v(64) | ones | 7 pad cols never touched]
EPS = 1e-5

_CACHE: dict = {}
LAST_RUN_NS: int | None = None


def _ilv(k):
    """Row shuffle so that SBUF [P, k, n] loaded with "(p k) n -> p k n"
    (contiguous k*rowbytes per partition) holds orig row 128*c+p at
    (partition p, chunk c):  shuf[k*p + c] = orig[128*c + p]."""
    return (np.arange(k)[None, :] * 128 + np.arange(128)[:, None]).reshape(-1)


IDX8 = _ilv(8)
IDX4 = _ilv(4)
IDX32 = _ilv(32)


def _bcast_row_ap(dram_ap, parts=P):
    """DRAM [1, N] -> partition-broadcast AP [parts, N] (step-0 partition dim)."""
    return bass.AP(
        tensor=dram_ap.tensor, offset=dram_ap.offset, ap=[[0, parts], dram_ap.ap[1]]
    )


def _build(flags: frozenset, repeat: int = 1, ablate: frozenset = frozenset()):
    """Build + compile the single SPMD program. `flags` toggles optional ops."""
    use_ca_mask = "ca_mask" in flags
    use_sa_full = "sa_full" in flags
    ln_g = {i: f"ln{i}_g" in flags for i in (1, 2, 3)}
    ln_b = {i: f"ln{i}_b" in flags for i in (1, 2, 3)}
    use_b2 = "b2" in flags

    nc = bacc.Bacc("TRN2", target_bir_lowering=False, debug=False, num_devices=8)

    D = {}

    def din(name, shape, dt):
        D[name] = nc.dram_tensor(name, shape, dt, kind="ExternalInput").ap()

    din("x_own", [QN, H], F16)
    din("x_rm", [S, H], F16)  # permuted rows (own first)
    din("kv_rm", [S, H], F16)
    din("sa_cb", [P, RC, QN], F16)  # additive causal bias, permuted key order
    if use_ca_mask:
        din("ca_cb", [P, RC, QN], F16)
    for w in ("sa_wq", "sa_wk", "sa_wv", "sa_wo", "ca_wq", "ca_wk", "ca_wv", "ca_wo"):
        din(w, [H, H], F16)
    din("fc_w1", [H, FF], F16)
    din("fc_w2", [FF, H], F16)
    din("fc_b1f", [P, FT], F32)  # feature-major b1
    if use_b2:
        din("fc_b2", [1, H], F32)
    for i in (1, 2, 3):
        if ln_g[i]:
            din(f"ln{i}_g", [1, H], F32)
        if ln_b[i]:
            din(f"ln{i}_b", [1, H], F32)
    out_d = nc.dram_tensor("out", [QN, H], F32, kind="ExternalOutput").ap()

    with tile.TileContext(nc) as tc, ExitStack() as top:
        sing = top.enter_context(tc.tile_pool(name="sing", bufs=1))
        ident = sing.tile([P, P], F16)
        make_identity(nc, ident)
        eps_t = sing.tile([P, 1], F32)
        nc.vector.memset(eps_t, EPS)
        ones_c = sing.tile([P, 1], F32)
        nc.vector.memset(ones_c, 1.0)

        # broadcast [1,H] params across partitions via step-0 DMA
        gb_tiles = {}
        for i in (1, 2, 3):
            for kind, on in (("g", ln_g[i]), ("b", ln_b[i])):
                if on:
                    t = sing.tile([P, H], F32)
                    nc.sync.dma_start(out=t, in_=_bcast_row_ap(D[f"ln{i}_{kind}"]))
                    gb_tiles[(i, kind)] = t
        b1f = sing.tile([P, FT], F32)
        nc.sync.dma_start(out=b1f, in_=D["fc_b1f"])
        b2_t = None
        if use_b2:
            b2_t = sing.tile([P, H], F32)
            nc.sync.dma_start(out=b2_t, in_=_bcast_row_ap(D["fc_b2"]))

        small = top.enter_context(tc.tile_pool(name="small", bufs=4))
        norms = top.enter_context(tc.tile_pool(name="norms", bufs=2))
        scratch = top.enter_context(tc.tile_pool(name="scratch", bufs=2))
        lnzp = top.enter_context(tc.tile_pool(name="lnzp", bufs=1))
        kvsrc = top.enter_context(tc.tile_pool(name="kvsrc", bufs=1))

        # persistent q tile: the zero halves (the head-masking mechanism for
        # the padded-contraction score matmuls) are written once here and
        # never touched again; SA and CA overwrite only their data rows.
        qTp = sing.tile([P, NH, QN], F16, tag="qTp")
        nc.vector.memset(qTp, 0.0)

        def emit_once():
         xown16 = sing.tile([P, QC, H], F16, tag="xown")
         nc.sync.dma_start(
             out=xown16, in_=D["x_own"].rearrange("(p q) n -> p q n", p=P)
         )
         sacb = sing.tile([P, RC, QN], F16, tag="sacb")
         nc.sync.dma_start(out=sacb, in_=D["sa_cb"])
         cacb = None
         if use_ca_mask:
             cacb = sing.tile([P, RC, QN], F16, tag="cacb")
             nc.sync.dma_start(out=cacb, in_=D["ca_cb"])
         z = sing.tile([P, QC, H], F16, tag="z")
         z2 = sing.tile([P, QC, H], F16, tag="z2")

         mm_stack = ExitStack()
         ps_mm = mm_stack.enter_context(
             tc.tile_pool(name="ps_mm", bufs=2, space="PSUM")
         )

         def layernorm_T(row_ap_fn, n_rc, lnT, ln_idx, ps_t, tag):
            """Row-major LN stats+apply, then PE-transpose into feature-major lnT.

            row_ap_fn(rc) -> SBUF AP [P, H] holding rows rc*128..+128.
            lnT: [P, HT, n_rc*P] fp16 tile (feat-in-tile, feat-tile, row).
            """
            for rc in range(n_rc):
                row = row_ap_fn(rc)
                st = small.tile([P, 2, 6], F32, tag="st")
                nc.vector.bn_stats(out=st[:, 0], in_=row[:, 0:512])
                nc.vector.bn_stats(out=st[:, 1], in_=row[:, 512:H])
                mv = small.tile([P, 2], F32, tag="mv")
                nc.vector.bn_aggr(out=mv, in_=st)
                # rstd = exp(-0.5*ln(var+eps)): keeps ACT on the one table set
                # that also serves Exp/Copy/Relu (no 2.7us table reloads), and
                # frees DVE of the reciprocal.
                lnv = small.tile([P, 1], F32, tag="lnv")
                nc.scalar.activation(
                    out=lnv, in_=mv[:, 1:2], func=AF.Ln, bias=eps_t, scale=1.0
                )
                rstd = small.tile([P, 1], F32, tag="rstd")
                nc.scalar.activation(out=rstd, in_=lnv, func=AF.Exp, scale=-0.5)
                lnr = scratch.tile([P, H], F16, tag="lnr")
                nc.vector.tensor_scalar(
                    out=lnr,
                    in0=row,
                    scalar1=mv[:, 0:1],
                    scalar2=rstd,
                    op0=OP.subtract,
                    op1=OP.mult,
                )
                if (ln_idx, "g") in gb_tiles:
                    nc.vector.tensor_mul(out=lnr, in0=lnr, in1=gb_tiles[(ln_idx, "g")])
                if (ln_idx, "b") in gb_tiles:
                    nc.vector.tensor_add(out=lnr, in0=lnr, in1=gb_tiles[(ln_idx, "b")])
                for f in range(HT):
                    pt = ps_t.tile([P, P], F16, tag="pt")
                    nc.tensor.transpose(pt, lnr[:, f * P : (f + 1) * P], ident)
                    # evict on ACT: DVE is busy with LN stats/apply in these
                    # phases while ACT is idle
                    nc.scalar.copy(
                        out=lnT[:, f, rc * P : (rc + 1) * P], in_=pt
                    )

         _w8_cache = {}

         def load_w8(dram, pool):
            if "nowdma" in ablate:
                if "w8" not in _w8_cache:
                    t = pool.tile([P, HT, H], F16, tag="w8")
                    nc.sync.dma_start(
                        out=t, in_=dram.rearrange("(p k) n -> p k n", p=P)
                    )
                    _w8_cache["w8"] = t
                return _w8_cache["w8"]
            t = pool.tile([P, HT, H], F16, tag="w8")
            nc.sync.dma_start(out=t, in_=dram.rearrange("(p k) n -> p k n", p=P))
            return t

         FULL_SCHED = ((512, 0),) * RC
         # uniform causally-clipped SA schedule (see _prep_core block layout):
         # kc pairs alternate full-width and upper-half-only (q cols 256..511)
         SA_SCHED = (
             (512, 0), (512, 0), (256, 256), (256, 256),
             (512, 0), (512, 0), (256, 256), (256, 256),
         )

         def attention(qT, kT, v_aug, ctxT, cb, ps_s, ps_av, exp_pool, sched):
            for h in range(NH):
                f, r0 = h // 2, (h % 2) * 64
                pm_av = (
                    None
                    if "noav" in ablate
                    else ps_av.tile([P, QN], F32, tag="av")
                )
                av_out = None if pm_av is None else pm_av[0:65]
                for g in range(4):
                    n, off = sched[2 * g]  # both kc of a pair share (n, off)
                    et = exp_pool.tile([P, 2, QN], F16, tag="et")
                    if "noscores" in ablate:
                        nc.vector.memset(et, 0.01)
                    else:
                        pm_s = ps_s.tile([P, 2, QN], F32, tag="s")
                        for j in range(2):
                            kc = g * 2 + j
                            # K=128 contraction: other head's rows in qT are 0
                            nc.tensor.matmul(
                                pm_s[:, j, 0:n],
                                lhsT=kT[:, f, kc * P : (kc + 1) * P],
                                rhs=qT[:, h, off : off + n],
                                start=True,
                                stop=True,
                            )
                        if "noexp" in ablate:
                            nc.vector.tensor_copy(
                                out=et[:, :, 0:n], in_=pm_s[:, :, 0:n]
                            )
                        else:
                            nc.scalar.activation(
                                out=et[:, :, 0:n],
                                in_=pm_s[:, :, 0:n],
                                func=AF.Exp,
                                scale=0.125,
                            )
                        if cb is not None and "nobias" not in ablate:
                            # exp(s/8)*m == masked softmax numerator (m in {0,1})
                            nc.vector.tensor_mul(
                                out=et[:, :, 0:n],
                                in0=et[:, :, 0:n],
                                in1=cb[:, 2 * g : 2 * g + 2, off : off + n],
                            )
                    if "noav" not in ablate:
                        for j in range(2):
                            kc = g * 2 + j
                            # lhsT = [v(64) | ones]: psum row 64 accumulates
                            # sumexp; only psum rows 0:65 are written/read.
                            nc.tensor.matmul(
                                av_out[:, off : off + n],
                                lhsT=v_aug[:, kc, h, 0:65],
                                rhs=et[:, j, 0:n],
                                start=(kc == 0),
                                stop=(kc == RC - 1),
                            )
                if "noav" in ablate:
                    nc.vector.memset(ctxT[r0 : r0 + 64, f, :], 0.25)
                elif "nonorm" in ablate:
                    nc.vector.tensor_copy(
                        out=ctxT[r0 : r0 + 64, f, :], in_=pm_av[0:64, :]
                    )
                else:
                    recip = norms.tile([1, QN], F16, tag="rec")
                    with nc.allow_low_precision(
                        reason="1/sumexp in fp16 is within output tolerance"
                    ):
                        nc.vector.reciprocal(out=recip, in_=pm_av[64:65, :])
                    rb = norms.tile([64, QN], F16, tag="rb")
                    nc.gpsimd.partition_broadcast(out_ap=rb, in_ap=recip)
                    nc.vector.tensor_mul(
                        out=ctxT[r0 : r0 + 64, f, :], in0=pm_av[0:64, :], in1=rb
                    )

         def proj_heads_qpad(qT_pad, w_sb, lnT):
            # qT_pad[:, h, :]: head h q-dims at rows (h%2)*64..+64 (matching its
            # row range inside the packed kT tile f=h//2), other 64 rows zero
            # (zeroed once at build time -- qT_pad is the persistent qTp).
            for f in range(HT):
                pm = ps_mm.tile([P, 512], F32, tag="proj")
                for kc in range(HT):
                    nc.tensor.matmul(
                        pm,
                        lhsT=w_sb[:, kc, f * P : (f + 1) * P],
                        rhs=lnT[:, kc, 0:QN],
                        start=(kc == 0),
                        stop=(kc == HT - 1),
                    )
                nc.vector.tensor_copy(out=qT_pad[0:64, 2 * f, :], in_=pm[0:64, :])
                nc.vector.tensor_copy(
                    out=qT_pad[64:128, 2 * f + 1, :], in_=pm[64:128, :]
                )

         def proj_to_featmajor(outT, w_sb, lnT, n_cols):
            # outT[:, f, c*512:+512] = sum_kc w[kc,f]^T @ lnT[kc, cols]
            for f in range(HT):
                for c in range(n_cols // 512):
                    pm = ps_mm.tile([P, 512], F32, tag="proj")
                    for kc in range(HT):
                        nc.tensor.matmul(
                            pm,
                            lhsT=w_sb[:, kc, f * P : (f + 1) * P],
                            rhs=lnT[:, kc, c * 512 : (c + 1) * 512],
                            start=(kc == 0),
                            stop=(kc == HT - 1),
                        )
                    nc.vector.tensor_copy(
                        out=outT[:, f, c * 512 : (c + 1) * 512], in_=pm
                    )

         def make_v_aug(v_aug, w_sb, lnT):
            # cols 65:VW are never written nor read (AV lhsT slices 0:65)
            nc.vector.tensor_copy(
                out=v_aug[:, :, :, 64:65], in_=ones_c.to_broadcast([P, RC, NH, 1])
            )
            for kc in range(RC):
                for vc in range(2):
                    pm = ps_mm.tile([P, 512], F32, tag="proj")
                    for hc in range(HT):
                        nc.tensor.matmul(
                            pm,
                            lhsT=lnT[:, hc, kc * P : (kc + 1) * P],
                            rhs=w_sb[:, hc, vc * 512 : (vc + 1) * 512],
                            start=(hc == 0),
                            stop=(hc == HT - 1),
                        )
                    nc.vector.tensor_copy(
                        out=v_aug[:, kc, vc * 8 : (vc + 1) * 8, 0:64],
                        in_=pm.rearrange("p (h d) -> p h d", h=8),
                    )

         def wo_residual(ctxT, w_sb, base, out_rows):
            # out_rows[:, qc, :] = base[:, qc, :] + ctx @ wo
            for qc in range(QC):
                for ncol in range(2):
                    pm = ps_mm.tile([P, 512], F32, tag="proj")
                    for hd in range(HT):
                        nc.tensor.matmul(
                            pm,
                            lhsT=ctxT[:, hd, qc * P : (qc + 1) * P],
                            rhs=w_sb[:, hd, ncol * 512 : (ncol + 1) * 512],
                            start=(hd == 0),
                            stop=(hd == HT - 1),
                        )
                    sl = slice(ncol * 512, (ncol + 1) * 512)
                    nc.vector.tensor_tensor(
                        out=out_rows[:, qc, sl], in0=pm, in1=base[:, qc, sl], op=OP.add
                    )

         lnz = lnzp.tile([P, HT, QN], F16, tag="lnzT")

         with tc.tile_pool(name="attn_acts", bufs=1) as acts, tc.tile_pool(
            name="wpool", bufs=2
         ) as wpool:
            # ---------------- P0: LN1(x) -> ln1T, LN2(kv) -> ln2kvT ----------
            # split DMAs so LN of the first half starts at half-transfer
            x_sb = acts.tile([P, RC, H], F16, tag="kT")
            x_ap = D["x_rm"].rearrange("(p k) n -> p k n", p=P)
            nc.sync.dma_start(out=x_sb[:, 0:4, :], in_=x_ap[:, 0:4, :])
            nc.sync.dma_start(out=x_sb[:, 4:8, :], in_=x_ap[:, 4:8, :])
            ln1T = acts.tile([P, HT, S], F16, tag="lnT")
            # kv LN is independent of everything up to CA: emit it here so it
            # fills DVE/ACT while the SA projections own PE
            kv_sb = kvsrc.tile([P, RC, H], F16, tag="kvrows")
            kv_ap = D["kv_rm"].rearrange("(p k) n -> p k n", p=P)
            nc.sync.dma_start(out=kv_sb[:, 0:4, :], in_=kv_ap[:, 0:4, :])
            nc.sync.dma_start(out=kv_sb[:, 4:8, :], in_=kv_ap[:, 4:8, :])
            ln2kvT = acts.tile([P, HT, S], F16, tag="lnT2")
            with tc.tile_pool(name="ps_t1", bufs=2, space="PSUM") as ps_t:
                layernorm_T(lambda rc: x_sb[:, rc, :], RC, ln1T, 1, ps_t, "l1")
                layernorm_T(lambda rc: kv_sb[:, rc, :], RC, ln2kvT, 2, ps_t, "l2kv")

            # ---------------- P1: SA projections ----------------
            qT = qTp
            kT = acts.tile([P, HT, S], F16, tag="kT")
            v_aug = acts.tile([P, RC, NH, VW], F16, tag="vaug")
            wq = load_w8(D["sa_wq"], wpool)
            proj_heads_qpad(qT, wq, ln1T)
            wk = load_w8(D["sa_wk"], wpool)
            proj_to_featmajor(kT, wk, ln1T, S)
            wv = load_w8(D["sa_wv"], wpool)
            make_v_aug(v_aug, wv, ln1T)

            # ---------------- SA attention ----------------
            ctxT = acts.tile([P, HT, QN], F16, tag="ctxT")
            if "noattn" in ablate:
                nc.vector.memset(ctxT, 0.25)
            else:
                with (
                    tc.tile_pool(name="ps_s1", bufs=2, space="PSUM") as ps_s,
                    tc.tile_pool(name="ps_av1", bufs=2, space="PSUM") as ps_av,
                    tc.tile_pool(name="exp1", bufs=3) as exp_pool,
                ):
                    attention(
                     qT, kT, v_aug, ctxT, sacb, ps_s, ps_av, exp_pool,
                     FULL_SCHED if use_sa_full else SA_SCHED,
                 )

            # ---------------- SA wo + residual -> z ----------------
            wo = load_w8(D["sa_wo"], wpool)
            wo_residual(ctxT, wo, xown16, z)

            # ---------------- P2: cross attention ----------------
            # CA K/V projections depend only on ln2kvT (ready since P0) and
            # can fill PE while SA attention is ACT(exp)-bound; kT2/v_aug2
            # reuse SA slots so scheduling overlap is limited to what WAR
            # hazards allow.
            cwk = load_w8(D["ca_wk"], wpool)
            kT2 = acts.tile([P, HT, S], F16, tag="kT")
            proj_to_featmajor(kT2, cwk, ln2kvT, S)
            cwv = load_w8(D["ca_wv"], wpool)
            v_aug2 = acts.tile([P, RC, NH, VW], F16, tag="vaug")
            make_v_aug(v_aug2, cwv, ln2kvT)

            with tc.tile_pool(name="ps_t2", bufs=2, space="PSUM") as ps_t:
                layernorm_T(lambda rc: z[:, rc, :], QC, lnz, 2, ps_t, "l2z")
            qT2 = qTp
            cwq = load_w8(D["ca_wq"], wpool)
            proj_heads_qpad(qT2, cwq, lnz)

            ctxT2 = acts.tile([P, HT, QN], F16, tag="ctxT")
            if "noattn" in ablate:
                nc.vector.memset(ctxT2, 0.25)
            else:
                with (
                    tc.tile_pool(name="ps_s2", bufs=2, space="PSUM") as ps_s,
                    tc.tile_pool(name="ps_av2", bufs=2, space="PSUM") as ps_av,
                    tc.tile_pool(name="exp2", bufs=3) as exp_pool,
                ):
                    attention(qT2, kT2, v_aug2, ctxT2, cacb, ps_s, ps_av, exp_pool, FULL_SCHED)

            cwo = load_w8(D["ca_wo"], wpool)
            wo_residual(ctxT2, cwo, z, z2)

         # ---------------- P3: FFN ----------------
         with tc.tile_pool(name="ps_t3", bufs=2, space="PSUM") as ps_t:
            layernorm_T(lambda rc: z2[:, rc, :], QC, lnz, 3, ps_t, "l3")

         with (
             tc.tile_pool(name="hTpool", bufs=1) as hTpool,
             tc.tile_pool(name="w2pool", bufs=3) as w2pool,
         ):
            hT = hTpool.tile([P, FT, QN], F16, tag="hT")
            w2_ap = D["fc_w2"].rearrange("(p k) n -> p k n", p=P)
            with tc.tile_pool(name="w1pool", bufs=1) as w1pool:
                # two halves so hT matmuls start after the first 4MB lands
                w1a = w1pool.tile([P, 4, FF], F16, tag="w1a")
                nc.sync.dma_start(
                    out=w1a,
                    in_=D["fc_w1"].rearrange("(p k) n -> p k n", p=P)[:, 0:4, :],
                )
                w1b = w1pool.tile([P, 4, FF], F16, tag="w1b")
                nc.sync.dma_start(
                    out=w1b,
                    in_=D["fc_w1"].rearrange("(p k) n -> p k n", p=P)[:, 4:8, :],
                )
                for ft in range(FT):
                    pm = ps_mm.tile([P, 512], F32, tag="proj")
                    for kc in range(HT):
                        w1t = w1a if kc < 4 else w1b
                        nc.tensor.matmul(
                            pm,
                            lhsT=w1t[:, kc % 4, ft * P : (ft + 1) * P],
                            rhs=lnz[:, kc, :],
                            start=(kc == 0),
                            stop=(kc == HT - 1),
                        )
                    nc.scalar.activation(
                        out=hT[:, ft, :],
                        in_=pm,
                        func=AF.Relu,
                        bias=b1f[:, ft : ft + 1],
                    )

            mm_stack.close()  # free ps_mm banks for ps_big
            out_rows = sing.tile([P, QC, H], F32, tag="z")  # reuses z slot
            with tc.tile_pool(name="ps_big", bufs=1, space="PSUM") as ps_big:
                pm8 = ps_big.tile([P, 8, 512], F32)
                w2t = None
                for kc in range(FT):
                    if kc % 4 == 0:
                        w2t = w2pool.tile([P, 4, H], F16, tag="w2s")
                        nc.sync.dma_start(
                            out=w2t, in_=w2_ap[:, kc : kc + 4, :]
                        )
                    for qc in range(QC):
                        for ncol in range(2):
                            nc.tensor.matmul(
                                pm8[:, qc * 2 + ncol, :],
                                lhsT=hT[:, kc, qc * P : (qc + 1) * P],
                                rhs=w2t[:, kc % 4, ncol * 512 : (ncol + 1) * 512],
                                start=(kc == 0),
                                stop=(kc == FT - 1),
                            )
                out_ap = out_d.rearrange("(q p) n -> p q n", p=P)
                for qc in range(QC):
                    for ncol in range(2):
                        sl = slice(ncol * 512, (ncol + 1) * 512)
                        nc.vector.tensor_tensor(
                            out=out_rows[:, qc, sl],
                            in0=pm8[:, qc * 2 + ncol, :],
                            in1=z2[:, qc, sl],
                            op=OP.add,
                        )
                        if b2_t is not None:
                            nc.vector.tensor_add(
                                out=out_rows[:, qc, sl],
                                in0=out_rows[:, qc, sl],
                                in1=b2_t[:, sl],
                            )
                    # stream each query chunk out as soon as it is complete
                    nc.sync.dma_start(
                        out=out_ap[:, qc : qc + 1, :],
                        in_=out_rows[:, qc : qc + 1, :],
                    )

        for _ in range(repeat):
            emit_once()

    nc.compile()
    return nc


def own_rows(half):
    """Query rows of a core: blocks {B0,B3} / {B1,B2} of 256 rows each.
    This interleaving is what makes the uniform SA_SCHED causally valid
    for both cores of a batch pair."""
    if half == 0:
        return np.concatenate([np.arange(0, 256), np.arange(768, 1024)])
    return np.arange(256, 768)


def _prep_core(c, x, kv, future_mask, mask, use_ca_mask):
    b, half = c // 2, c % 2
    own = own_rows(half)
    if half == 0:
        rest = np.concatenate([np.arange(256, 512), np.arange(512, 768)])
    else:
        rest = np.concatenate([np.arange(0, 256), np.arange(768, 1024)])
    perm = np.concatenate([own, rest])
    m = {}
    m["x_own"] = np.ascontiguousarray(x[b, own][IDX4]).astype(np.float16)
    m["x_rm"] = np.ascontiguousarray(x[b][perm][IDX8]).astype(np.float16)
    m["kv_rm"] = np.ascontiguousarray(kv[b][IDX8]).astype(np.float16)
    # sa_cb[p, kc, q] = 0 where future_mask[b, own_q, perm_key] else 1 (key=kc*128+p)
    fm = future_mask[b, own][:, perm]  # [QN, S] bool
    cb = np.where(fm.T, np.float16(0.0), np.float16(1.0))  # [S, QN]
    m["sa_cb"] = np.ascontiguousarray(cb.reshape(RC, P, QN).transpose(1, 0, 2))
    if use_ca_mask:
        cm = mask[b, own]  # [QN, S]
        ccb = np.where(cm.T, np.float16(0.0), np.float16(1.0))
        m["ca_cb"] = np.ascontiguousarray(ccb.reshape(RC, P, QN).transpose(1, 0, 2))
    return m


def _prep_shared(inp):
    shared = {}
    for w in ("sa_wq", "sa_wk", "sa_wv", "sa_wo", "ca_wq", "ca_wk", "ca_wv", "ca_wo"):
        shared[w] = np.ascontiguousarray(np.asarray(inp[w])[IDX8]).astype(np.float16)
    shared["fc_w1"] = np.ascontiguousarray(
        np.asarray(inp["fc_w1"])[IDX8]
    ).astype(np.float16)
    shared["fc_w2"] = np.ascontiguousarray(
        np.asarray(inp["fc_w2"])[IDX32]
    ).astype(np.float16)
    shared["fc_b1f"] = np.ascontiguousarray(
        np.asarray(inp["fc_b1"]).reshape(FT, P).T
    ).astype(np.float32)
    return shared


def kernel(**inputs) -> np.ndarray:
    global LAST_RUN_NS
    inp = {k: np.asarray(v) for k, v in inputs.items()}
    x, kv = inp["x"], inp["key_and_value"]
    mask, future_mask = inp["mask"], inp["future_mask"]

    flags = set()
    if mask.any():
        flags.add("ca_mask")
    # The clipped SA_SCHED structurally skips regions that a standard causal
    # mask guarantees are masked.  Only safe if future_mask IS causal triu;
    # otherwise fall back to the full-rectangle schedule (mask data covers it).
    tri = np.triu(np.ones((S, S), dtype=bool), 1)
    if not all(np.array_equal(future_mask[b], tri) for b in range(B)):
        flags.add("sa_full")
    for i in (1, 2, 3):
        if not np.all(inp[f"ln{i}_g"] == 1.0):
            flags.add(f"ln{i}_g")
        if np.any(inp[f"ln{i}_b"] != 0.0):
            flags.add(f"ln{i}_b")
    if np.any(inp["fc_b2"] != 0.0):
        flags.add("b2")
    flags = frozenset(flags)

    if flags not in _CACHE:
        _CACHE[flags] = _build(flags)
    nc = _CACHE[flags]

    shared = _prep_shared(inp)
    if "b2" in flags:
        shared["fc_b2"] = inp["fc_b2"].reshape(1, H).astype(np.float32)
    for i in (1, 2, 3):
        if f"ln{i}_g" in flags:
            shared[f"ln{i}_g"] = inp[f"ln{i}_g"].reshape(1, H).astype(np.float32)
        if f"ln{i}_b" in flags:
            shared[f"ln{i}_b"] = inp[f"ln{i}_b"].reshape(1, H).astype(np.float32)

    in_maps = []
    for c in range(8):
        m = _prep_core(c, x, kv, future_mask, mask, "ca_mask" in flags)
        m.update(shared)
        in_maps.append(m)

    from concourse import bass_utils

    t0 = time.perf_counter_ns()
    res = bass_utils.run_bass_kernel_spmd(
        nc, in_maps, core_ids=list(range(8)), trace=False
    )
    LAST_RUN_NS = time.perf_counter_ns() - t0

    out = np.empty((B, S, H), np.float32)
    for c in range(8):
        b, half = c // 2, c % 2
        out[b, own_rows(half)] = res.results[c]["out"]
    return out



# revision 15
# speedup vs baseline: 20.5396x; 2.5709x over previous
"""Trainium2 Bass kernel for nn_DecoderBlock (B=4, S=1024, H=1024, 16 heads).

Sharding (8 cores, zero cross-core communication):
  core c -> batch b = c//2, half = c%2; own query rows are the interleaved
  256-row blocks {B0,B3} (half 0) / {B1,B2} (half 1) -- see own_rows().
  Row-parallel over the sequence for LN / projections / FFN; each core of a
  batch pair duplicates the full K/V projections (they depend only on the
  INPUTS x / key_and_value, never on the other core's partial results).

Device layout strategy:
  - fp16 operands for every matmul (1 cyc/row on PE), fp32 PSUM accumulate.
  - Attention in transposed-score space: scoresT[k, q] = kT.T @ qT per head,
    exp on ACT (scale=1/8 folded in), causal/padding mask as a 0/1 fp16
    MULTIPLY on the exp output (host-derived from the boolean masks; exact),
    softmax denominator via a ones-column appended to V (one extra PSUM row),
    normalized with reciprocal + gpsimd partition_broadcast.  ctxT feeds the
    output projection directly as lhsT - attention is never transposed.
  - Rows are host-permuted own-first so the uniform SPMD program slices "my
    queries" at column 0; softmax attention is key-order invariant and the
    mask tiles are built in permuted key order.  The interleaved block
    sharding makes the causally-clipped SA_SCHED (75% of the full rectangle,
    alternating full-width / upper-half-only key-chunk pairs) valid for BOTH
    cores of a pair with one uniform program; data masks cover the rest.
"""

import sys

sys.path.insert(0, "/opt/trn_rl_repo")

import time
from contextlib import ExitStack

import numpy as np

import concourse.bass as bass
import concourse.mybir as mybir
import concourse.tile as tile
from concourse import bacc
from concourse.masks import make_identity

F32 = mybir.dt.float32
F16 = mybir.dt.float16
AF = mybir.ActivationFunctionType
OP = mybir.AluOpType

B, S, H, NH, DK, FF = 4, 1024, 1024, 16, 64, 4096
P = 128
HT = H // P  # 8 feature tiles of the model dim
QN = 512  # own query rows per core
QC = QN // P  # 4 query chunks
RC = S // P  # 8 key/row chunks
FT = FF // P  # 32 ffn tiles
NEG = -50000.0  # fp16-safe -inf surrogate (exp(NEG/8) == 0 in fp32)
VW = 72  # per-head width of v_aug: [v(64) | ones | 7 pad cols never touched]
EPS = 1e-5

_CACHE: dict = {}
LAST_RUN_NS: int | None = None


def _ilv(k):
    """Row shuffle so that SBUF [P, k, n] loaded with "(p k) n -> p k n"
    (contiguous k*rowbytes per partition) holds orig row 128*c+p at
    (partition p, chunk c):  shuf[k*p + c] = orig[128*c + p]."""
    return (np.arange(k)[None, :] * 128 + np.arange(128)[:, None]).reshape(-1)


IDX8 = _ilv(8)
IDX4 = _ilv(4)
IDX32 = _ilv(32)


def _bcast_row_ap(dram_ap, parts=P):
    """DRAM [1, N] -> partition-broadcast AP [parts, N] (step-0 partition dim)."""
    return bass.AP(
        tensor=dram_ap.tensor, offset=dram_ap.offset, ap=[[0, parts], dram_ap.ap[1]]
    )


def _build(flags: frozenset, repeat: int = 1, ablate: frozenset = frozenset()):
    """Build + compile the single SPMD program. `flags` toggles optional ops."""
    use_ca_mask = "ca_mask" in flags
    use_sa_full = "sa_full" in flags
    ln_g = {i: f"ln{i}_g" in flags for i in (1, 2, 3)}
    ln_b = {i: f"ln{i}_b" in flags for i in (1, 2, 3)}
    use_b2 = "b2" in flags

    nc = bacc.Bacc("TRN2", target_bir_lowering=False, debug=False, num_devices=8)

    D = {}

    def din(name, shape, dt):
        D[name] = nc.dram_tensor(name, shape, dt, kind="ExternalInput").ap()

    din("x_own", [QN, H], F16)
    din("x_rm", [S, H], F16)  # permuted rows (own first)
    din("kv_rm", [S, H], F16)
    din("sa_cb", [P, RC, QN], F16)  # additive causal bias, permuted key order
    if use_ca_mask:
        din("ca_cb", [P, RC, QN], F16)
    for w in ("sa_wq", "sa_wk", "sa_wv", "sa_wo", "ca_wq", "ca_wk", "ca_wv", "ca_wo"):
        din(w, [H, H], F16)
    din("fc_w1", [H, FF], F16)
    din("fc_w2", [FF, H], F16)
    din("fc_b1f", [P, FT], F32)  # feature-major b1
    if use_b2:
        din("fc_b2", [1, H], F32)
    for i in (1, 2, 3):
        if ln_g[i]:
            din(f"ln{i}_g", [1, H], F32)
        if ln_b[i]:
            din(f"ln{i}_b", [1, H], F32)
    out_d = nc.dram_tensor("out", [QN, H], F32, kind="ExternalOutput").ap()

    with tile.TileContext(nc) as tc, ExitStack() as top:
        sing = top.enter_context(tc.tile_pool(name="sing", bufs=1))
        ident = sing.tile([P, P], F16)
        make_identity(nc, ident)
        eps_t = sing.tile([P, 1], F32)
        nc.vector.memset(eps_t, EPS)
        ones_c = sing.tile([P, 1], F32)
        nc.vector.memset(ones_c, 1.0)

        # broadcast [1,H] params across partitions via step-0 DMA
        gb_tiles = {}
        for i in (1, 2, 3):
            for kind, on in (("g", ln_g[i]), ("b", ln_b[i])):
                if on:
                    t = sing.tile([P, H], F32)
                    nc.sync.dma_start(out=t, in_=_bcast_row_ap(D[f"ln{i}_{kind}"]))
                    gb_tiles[(i, kind)] = t
        b1f = sing.tile([P, FT], F32)
        nc.sync.dma_start(out=b1f, in_=D["fc_b1f"])
        b2_t = None
        if use_b2:
            b2_t = sing.tile([P, H], F32)
            nc.sync.dma_start(out=b2_t, in_=_bcast_row_ap(D["fc_b2"]))

        small = top.enter_context(tc.tile_pool(name="small", bufs=4))
        norms = top.enter_context(tc.tile_pool(name="norms", bufs=2))
        scratch = top.enter_context(tc.tile_pool(name="scratch", bufs=2))
        lnzp = top.enter_context(tc.tile_pool(name="lnzp", bufs=1))

        # persistent q tile: the zero halves (the head-masking mechanism for
        # the padded-contraction score matmuls) are written once here and
        # never touched again; SA and CA overwrite only their data rows.
        qTp = sing.tile([P, NH, QN], F16, tag="qTp")
        nc.vector.memset(qTp, 0.0)

        def emit_once():
         xown16 = sing.tile([P, QC, H], F16, tag="xown")
         nc.sync.dma_start(
             out=xown16, in_=D["x_own"].rearrange("(p q) n -> p q n", p=P)
         )
         sacb = sing.tile([P, RC, QN], F16, tag="sacb")
         nc.sync.dma_start(out=sacb, in_=D["sa_cb"])
         cacb = None
         if use_ca_mask:
             cacb = sing.tile([P, RC, QN], F16, tag="cacb")
             nc.sync.dma_start(out=cacb, in_=D["ca_cb"])
         z = sing.tile([P, QC, H], F16, tag="z")
         z2 = sing.tile([P, QC, H], F16, tag="z2")

         mm_stack = ExitStack()
         ps_mm = mm_stack.enter_context(
             tc.tile_pool(name="ps_mm", bufs=2, space="PSUM")
         )

         def layernorm_T(row_ap_fn, n_rc, lnT, ln_idx, ps_t, tag):
            """Row-major LN stats+apply, then PE-transpose into feature-major lnT.

            row_ap_fn(rc) -> SBUF AP [P, H] holding rows rc*128..+128.
            lnT: [P, HT, n_rc*P] fp16 tile (feat-in-tile, feat-tile, row).
            """
            for rc in range(n_rc):
                row = row_ap_fn(rc)
                st = small.tile([P, 2, 6], F32, tag="st")
                nc.vector.bn_stats(out=st[:, 0], in_=row[:, 0:512])
                nc.vector.bn_stats(out=st[:, 1], in_=row[:, 512:H])
                mv = small.tile([P, 2], F32, tag="mv")
                nc.vector.bn_aggr(out=mv, in_=st)
                # rstd = exp(-0.5*ln(var+eps)): keeps ACT on the one table set
                # that also serves Exp/Copy/Relu (no 2.7us table reloads), and
                # frees DVE of the reciprocal.
                lnv = small.tile([P, 1], F32, tag="lnv")
                nc.scalar.activation(
                    out=lnv, in_=mv[:, 1:2], func=AF.Ln, bias=eps_t, scale=1.0
                )
                rstd = small.tile([P, 1], F32, tag="rstd")
                nc.scalar.activation(out=rstd, in_=lnv, func=AF.Exp, scale=-0.5)
                lnr = scratch.tile([P, H], F16, tag="lnr")
                nc.vector.tensor_scalar(
                    out=lnr,
                    in0=row,
                    scalar1=mv[:, 0:1],
                    scalar2=rstd,
                    op0=OP.subtract,
                    op1=OP.mult,
                )
                if (ln_idx, "g") in gb_tiles:
                    nc.vector.tensor_mul(out=lnr, in0=lnr, in1=gb_tiles[(ln_idx, "g")])
                if (ln_idx, "b") in gb_tiles:
                    nc.vector.tensor_add(out=lnr, in0=lnr, in1=gb_tiles[(ln_idx, "b")])
                for f in range(HT):
                    pt = ps_t.tile([P, P], F16, tag="pt")
                    nc.tensor.transpose(pt, lnr[:, f * P : (f + 1) * P], ident)
                    # evict on ACT: DVE is busy with LN stats/apply in these
                    # phases while ACT is idle
                    nc.scalar.copy(
                        out=lnT[:, f, rc * P : (rc + 1) * P], in_=pt
                    )

         _w8_cache = {}

         def load_w8(dram, pool):
            if "nowdma" in ablate:
                if "w8" not in _w8_cache:
                    t = pool.tile([P, HT, H], F16, tag="w8")
                    nc.sync.dma_start(
                        out=t, in_=dram.rearrange("(p k) n -> p k n", p=P)
                    )
                    _w8_cache["w8"] = t
                return _w8_cache["w8"]
            t = pool.tile([P, HT, H], F16, tag="w8")
            nc.sync.dma_start(out=t, in_=dram.rearrange("(p k) n -> p k n", p=P))
            return t

         FULL_SCHED = ((512, 0),) * RC
         # uniform causally-clipped SA schedule (see _prep_core block layout):
         # kc pairs alternate full-width and upper-half-only (q cols 256..511)
         SA_SCHED = (
             (512, 0), (512, 0), (256, 256), (256, 256),
             (512, 0), (512, 0), (256, 256), (256, 256),
         )

         def attention(qT, kT, v_aug, ctxT, cb, ps_s, ps_av, exp_pool, sched):
            for h in range(NH):
                f, r0 = h // 2, (h % 2) * 64
                pm_av = (
                    None
                    if "noav" in ablate
                    else ps_av.tile([P, QN], F32, tag="av")
                )
                av_out = None if pm_av is None else pm_av[0:65]
                for g in range(4):
                    n, off = sched[2 * g]  # both kc of a pair share (n, off)
                    et = exp_pool.tile([P, 2, QN], F16, tag="et")
                    if "noscores" in ablate:
                        nc.vector.memset(et, 0.01)
                    else:
                        pm_s = ps_s.tile([P, 2, QN], F32, tag="s")
                        for j in range(2):
                            kc = g * 2 + j
                            # K=128 contraction: other head's rows in qT are 0
                            nc.tensor.matmul(
                                pm_s[:, j, 0:n],
                                lhsT=kT[:, f, kc * P : (kc + 1) * P],
                                rhs=qT[:, h, off : off + n],
                                start=True,
                                stop=True,
                            )
                        if "noexp" in ablate:
                            nc.vector.tensor_copy(
                                out=et[:, :, 0:n], in_=pm_s[:, :, 0:n]
                            )
                        else:
                            nc.scalar.activation(
                                out=et[:, :, 0:n],
                                in_=pm_s[:, :, 0:n],
                                func=AF.Exp,
                                scale=0.125,
                            )
                        if cb is not None and "nobias" not in ablate:
                            # exp(s/8)*m == masked softmax numerator (m in {0,1})
                            nc.vector.tensor_mul(
                                out=et[:, :, 0:n],
                                in0=et[:, :, 0:n],
                                in1=cb[:, 2 * g : 2 * g + 2, off : off + n],
                            )
                    if "noav" not in ablate:
                        for j in range(2):
                            kc = g * 2 + j
                            # lhsT = [v(64) | ones]: psum row 64 accumulates
                            # sumexp; only psum rows 0:65 are written/read.
                            nc.tensor.matmul(
                                av_out[:, off : off + n],
                                lhsT=v_aug[:, kc, h, 0:65],
                                rhs=et[:, j, 0:n],
                                start=(kc == 0),
                                stop=(kc == RC - 1),
                            )
                if "noav" in ablate:
                    nc.vector.memset(ctxT[r0 : r0 + 64, f, :], 0.25)
                elif "nonorm" in ablate:
                    nc.vector.tensor_copy(
                        out=ctxT[r0 : r0 + 64, f, :], in_=pm_av[0:64, :]
                    )
                else:
                    recip = norms.tile([1, QN], F16, tag="rec")
                    with nc.allow_low_precision(
                        reason="1/sumexp in fp16 is within output tolerance"
                    ):
                        nc.vector.reciprocal(out=recip, in_=pm_av[64:65, :])
                    rb = norms.tile([64, QN], F16, tag="rb")
                    nc.gpsimd.partition_broadcast(out_ap=rb, in_ap=recip)
                    nc.vector.tensor_mul(
                        out=ctxT[r0 : r0 + 64, f, :], in0=pm_av[0:64, :], in1=rb
                    )

         def proj_heads_qpad(qT_pad, w_sb, lnT):
            # qT_pad[:, h, :]: head h q-dims at rows (h%2)*64..+64 (matching its
            # row range inside the packed kT tile f=h//2), other 64 rows zero
            # (zeroed once at build time -- qT_pad is the persistent qTp).
            for f in range(HT):
                pm = ps_mm.tile([P, 512], F32, tag="proj")
                for kc in range(HT):
                    nc.tensor.matmul(
                        pm,
                        lhsT=w_sb[:, kc, f * P : (f + 1) * P],
                        rhs=lnT[:, kc, 0:QN],
                        start=(kc == 0),
                        stop=(kc == HT - 1),
                    )
                nc.vector.tensor_copy(out=qT_pad[0:64, 2 * f, :], in_=pm[0:64, :])
                nc.vector.tensor_copy(
                    out=qT_pad[64:128, 2 * f + 1, :], in_=pm[64:128, :]
                )

         def proj_to_featmajor(outT, w_sb, lnT, n_cols):
            # outT[:, f, c*512:+512] = sum_kc w[kc,f]^T @ lnT[kc, cols]
            for f in range(HT):
                for c in range(n_cols // 512):
                    pm = ps_mm.tile([P, 512], F32, tag="proj")
                    for kc in range(HT):
                        nc.tensor.matmul(
                            pm,
                            lhsT=w_sb[:, kc, f * P : (f + 1) * P],
                            rhs=lnT[:, kc, c * 512 : (c + 1) * 512],
                            start=(kc == 0),
                            stop=(kc == HT - 1),
                        )
                    nc.vector.tensor_copy(
                        out=outT[:, f, c * 512 : (c + 1) * 512], in_=pm
                    )

         def make_v_aug(v_aug, w_sb, lnT):
            # cols 65:VW are never written nor read (AV lhsT slices 0:65)
            nc.vector.tensor_copy(
                out=v_aug[:, :, :, 64:65], in_=ones_c.to_broadcast([P, RC, NH, 1])
            )
            for kc in range(RC):
                for vc in range(2):
                    pm = ps_mm.tile([P, 512], F32, tag="proj")
                    for hc in range(HT):
                        nc.tensor.matmul(
                            pm,
                            lhsT=lnT[:, hc, kc * P : (kc + 1) * P],
                            rhs=w_sb[:, hc, vc * 512 : (vc + 1) * 512],
                            start=(hc == 0),
                            stop=(hc == HT - 1),
                        )
                    nc.vector.tensor_copy(
                        out=v_aug[:, kc, vc * 8 : (vc + 1) * 8, 0:64],
                        in_=pm.rearrange("p (h d) -> p h d", h=8),
                    )

         def wo_residual(ctxT, w_sb, base, out_rows):
            # out_rows[:, qc, :] = base[:, qc, :] + ctx @ wo
            for qc in range(QC):
                for ncol in range(2):
                    pm = ps_mm.tile([P, 512], F32, tag="proj")
                    for hd in range(HT):
                        nc.tensor.matmul(
                            pm,
                            lhsT=ctxT[:, hd, qc * P : (qc + 1) * P],
                            rhs=w_sb[:, hd, ncol * 512 : (ncol + 1) * 512],
                            start=(hd == 0),
                            stop=(hd == HT - 1),
                        )
                    sl = slice(ncol * 512, (ncol + 1) * 512)
                    nc.vector.tensor_tensor(
                        out=out_rows[:, qc, sl], in0=pm, in1=base[:, qc, sl], op=OP.add
                    )

         lnz = lnzp.tile([P, HT, QN], F16, tag="lnzT")

         with tc.tile_pool(name="attn_acts", bufs=1) as acts, tc.tile_pool(
            name="wpool", bufs=2
         ) as wpool:
            # ---------------- P0: LN1(x) -> ln1T ----------------
            # quarter the DMA so LN of the first chunks starts early
            x_sb = acts.tile([P, RC, H], F16, tag="kT")
            x_ap = D["x_rm"].rearrange("(p k) n -> p k n", p=P)
            for i in range(4):
                nc.sync.dma_start(
                    out=x_sb[:, 2 * i : 2 * i + 2, :], in_=x_ap[:, 2 * i : 2 * i + 2, :]
                )
            ln1T = acts.tile([P, HT, S], F16, tag="lnT")
            kv_sb = acts.tile([P, RC, H], F16, tag="kvrows")
            kv_ap = D["kv_rm"].rearrange("(p k) n -> p k n", p=P)
            nc.sync.dma_start(out=kv_sb[:, 0:4, :], in_=kv_ap[:, 0:4, :])
            nc.sync.dma_start(out=kv_sb[:, 4:8, :], in_=kv_ap[:, 4:8, :])
            ln2kvT = acts.tile([P, HT, S], F16, tag="lnT2")
            with tc.tile_pool(name="ps_t1", bufs=2, space="PSUM") as ps_t:
                layernorm_T(lambda rc: x_sb[:, rc, :], RC, ln1T, 1, ps_t, "l1")

                # ------------- P1: SA projections -------------
                # kv LN is emitted between the projections: its DVE/ACT work
                # fills the PE-bound projection window (engines execute their
                # streams in emission order), keeping it off the critical path
                # well before CA needs it.
                qT = qTp
                kT = acts.tile([P, HT, S], F16, tag="kT")
                v_aug = acts.tile([P, RC, NH, VW], F16, tag="vaug")
                wq = load_w8(D["sa_wq"], wpool)
                proj_heads_qpad(qT, wq, ln1T)
                wk = load_w8(D["sa_wk"], wpool)
                proj_to_featmajor(kT, wk, ln1T, S)
                layernorm_T(lambda rc: kv_sb[:, rc, :], RC, ln2kvT, 2, ps_t, "l2kv")
                wv = load_w8(D["sa_wv"], wpool)
                make_v_aug(v_aug, wv, ln1T)

            # ---------------- SA attention ----------------
            ctxT = acts.tile([P, HT, QN], F16, tag="ctxT")
            if "noattn" in ablate:
                nc.vector.memset(ctxT, 0.25)
            else:
                with (
                    tc.tile_pool(name="ps_s1", bufs=2, space="PSUM") as ps_s,
                    tc.tile_pool(name="ps_av1", bufs=2, space="PSUM") as ps_av,
                    tc.tile_pool(name="exp1", bufs=3) as exp_pool,
                ):
                    attention(
                     qT, kT, v_aug, ctxT, sacb, ps_s, ps_av, exp_pool,
                     FULL_SCHED if use_sa_full else SA_SCHED,
                 )

            # ---------------- SA wo + residual -> z ----------------
            wo = load_w8(D["sa_wo"], wpool)
            wo_residual(ctxT, wo, xown16, z)

            # ---------------- P2: cross attention ----------------
            # CA K/V projections depend only on ln2kvT (ready since P0) and
            # can fill PE while SA attention is ACT(exp)-bound; kT2/v_aug2
            # reuse SA slots so scheduling overlap is limited to what WAR
            # hazards allow.
            cwk = load_w8(D["ca_wk"], wpool)
            kT2 = acts.tile([P, HT, S], F16, tag="kT")
            proj_to_featmajor(kT2, cwk, ln2kvT, S)
            cwv = load_w8(D["ca_wv"], wpool)
            v_aug2 = acts.tile([P, RC, NH, VW], F16, tag="vaug")
            make_v_aug(v_aug2, cwv, ln2kvT)

            with tc.tile_pool(name="ps_t2", bufs=2, space="PSUM") as ps_t:
                layernorm_T(lambda rc: z[:, rc, :], QC, lnz, 2, ps_t, "l2z")
            qT2 = qTp
            cwq = load_w8(D["ca_wq"], wpool)
            proj_heads_qpad(qT2, cwq, lnz)

            ctxT2 = acts.tile([P, HT, QN], F16, tag="ctxT")
            if "noattn" in ablate:
                nc.vector.memset(ctxT2, 0.25)
            else:
                with (
                    tc.tile_pool(name="ps_s2", bufs=2, space="PSUM") as ps_s,
                    tc.tile_pool(name="ps_av2", bufs=2, space="PSUM") as ps_av,
                    tc.tile_pool(name="exp2", bufs=3) as exp_pool,
                ):
                    attention(qT2, kT2, v_aug2, ctxT2, cacb, ps_s, ps_av, exp_pool, FULL_SCHED)

            cwo = load_w8(D["ca_wo"], wpool)
            wo_residual(ctxT2, cwo, z, z2)

         # ---------------- P3: FFN ----------------
         with tc.tile_pool(name="ps_t3", bufs=2, space="PSUM") as ps_t:
            layernorm_T(lambda rc: z2[:, rc, :], QC, lnz, 3, ps_t, "l3")

         with (
             tc.tile_pool(name="hTpool", bufs=1) as hTpool,
             tc.tile_pool(name="w2pool", bufs=3) as w2pool,
         ):
            hT = hTpool.tile([P, FT, QN], F16, tag="hT")
            w2_ap = D["fc_w2"].rearrange("(p k) n -> p k n", p=P)
            with tc.tile_pool(name="w1pool", bufs=1) as w1pool:
                # two halves so hT matmuls start after the first 4MB lands
                w1a = w1pool.tile([P, 4, FF], F16, tag="w1a")
                nc.sync.dma_start(
                    out=w1a,
                    in_=D["fc_w1"].rearrange("(p k) n -> p k n", p=P)[:, 0:4, :],
                )
                w1b = w1pool.tile([P, 4, FF], F16, tag="w1b")
                nc.sync.dma_start(
                    out=w1b,
                    in_=D["fc_w1"].rearrange("(p k) n -> p k n", p=P)[:, 4:8, :],
                )
                for ft in range(FT):
                    pm = ps_mm.tile([P, 512], F32, tag="proj")
                    for kc in range(HT):
                        w1t = w1a if kc < 4 else w1b
                        nc.tensor.matmul(
                            pm,
                            lhsT=w1t[:, kc % 4, ft * P : (ft + 1) * P],
                            rhs=lnz[:, kc, :],
                            start=(kc == 0),
                            stop=(kc == HT - 1),
                        )
                    nc.scalar.activation(
                        out=hT[:, ft, :],
                        in_=pm,
                        func=AF.Relu,
                        bias=b1f[:, ft : ft + 1],
                    )

            mm_stack.close()  # free ps_mm banks for ps_big
            out_rows = sing.tile([P, QC, H], F32, tag="z")  # reuses z slot
            with tc.tile_pool(name="ps_big", bufs=1, space="PSUM") as ps_big:
                pm8 = ps_big.tile([P, 8, 512], F32)
                w2t = None
                for kc in range(FT):
                    if kc % 4 == 0:
                        w2t = w2pool.tile([P, 4, H], F16, tag="w2s")
                        nc.sync.dma_start(
                            out=w2t, in_=w2_ap[:, kc : kc + 4, :]
                        )
                    for qc in range(QC):
                        for ncol in range(2):
                            nc.tensor.matmul(
                                pm8[:, qc * 2 + ncol, :],
                                lhsT=hT[:, kc, qc * P : (qc + 1) * P],
                                rhs=w2t[:, kc % 4, ncol * 512 : (ncol + 1) * 512],
                                start=(kc == 0),
                                stop=(kc == FT - 1),
                            )
                out_ap = out_d.rearrange("(q p) n -> p q n", p=P)
                for qc in range(QC):
                    for ncol in range(2):
                        sl = slice(ncol * 512, (ncol + 1) * 512)
                        nc.vector.tensor_tensor(
                            out=out_rows[:, qc, sl],
                            in0=pm8[:, qc * 2 + ncol, :],
                            in1=z2[:, qc, sl],
                            op=OP.add,
                        )
                        if b2_t is not None:
                            nc.vector.tensor_add(
                                out=out_rows[:, qc, sl],
                                in0=out_rows[:, qc, sl],
                                in1=b2_t[:, sl],
                            )
                    # stream each query chunk out as soon as it is complete
                    nc.sync.dma_start(
                        out=out_ap[:, qc : qc + 1, :],
                        in_=out_rows[:, qc : qc + 1, :],
                    )

        for _ in range(repeat):
            emit_once()

    nc.compile()
    return nc


def own_rows(half):
    """Query rows of a core: blocks {B0,B3} / {B1,B2} of 256 rows each.
    This interleaving is what makes the uniform SA_SCHED causally valid
    for both cores of a batch pair."""
    if half == 0:
        return np.concatenate([np.arange(0, 256), np.arange(768, 1024)])
    return np.arange(256, 768)


def _prep_core(c, x, kv, future_mask, mask, use_ca_mask):
    b, half = c // 2, c % 2
    own = own_rows(half)
    if half == 0:
        rest = np.concatenate([np.arange(256, 512), np.arange(512, 768)])
    else:
        rest = np.concatenate([np.arange(0, 256), np.arange(768, 1024)])
    perm = np.concatenate([own, rest])
    m = {}
    m["x_own"] = np.ascontiguousarray(x[b, own][IDX4]).astype(np.float16)
    m["x_rm"] = np.ascontiguousarray(x[b][perm][IDX8]).astype(np.float16)
    m["kv_rm"] = np.ascontiguousarray(kv[b][IDX8]).astype(np.float16)
    # sa_cb[p, kc, q] = 0 where future_mask[b, own_q, perm_key] else 1 (key=kc*128+p)
    fm = future_mask[b, own][:, perm]  # [QN, S] bool
    cb = np.where(fm.T, np.float16(0.0), np.float16(1.0))  # [S, QN]
    m["sa_cb"] = np.ascontiguousarray(cb.reshape(RC, P, QN).transpose(1, 0, 2))
    if use_ca_mask:
        cm = mask[b, own]  # [QN, S]
        ccb = np.where(cm.T, np.float16(0.0), np.float16(1.0))
        m["ca_cb"] = np.ascontiguousarray(ccb.reshape(RC, P, QN).transpose(1, 0, 2))
    return m


def _prep_shared(inp):
    shared = {}
    for w in ("sa_wq", "sa_wk", "sa_wv", "sa_wo", "ca_wq", "ca_wk", "ca_wv", "ca_wo"):
        shared[w] = np.ascontiguousarray(np.asarray(inp[w])[IDX8]).astype(np.float16)
    shared["fc_w1"] = np.ascontiguousarray(
        np.asarray(inp["fc_w1"])[IDX8]
    ).astype(np.float16)
    shared["fc_w2"] = np.ascontiguousarray(
        np.asarray(inp["fc_w2"])[IDX32]
    ).astype(np.float16)
    shared["fc_b1f"] = np.ascontiguousarray(
        np.asarray(inp["fc_b1"]).reshape(FT, P).T
    ).astype(np.float32)
    return shared


def kernel(**inputs) -> np.ndarray:
    global LAST_RUN_NS
    inp = {k: np.asarray(v) for k, v in inputs.items()}
    x, kv = inp["x"], inp["key_and_value"]
    mask, future_mask = inp["mask"], inp["future_mask"]

    flags = set()
    if mask.any():
        flags.add("ca_mask")
    # The clipped SA_SCHED structurally skips regions that a standard causal
    # mask guarantees are masked.  Only safe if future_mask IS causal triu;
    # otherwise fall back to the full-rectangle schedule (mask data covers it).
    tri = np.triu(np.ones((S, S), dtype=bool), 1)
    if not all(np.array_equal(future_mask[b], tri) for b in range(B)):
        flags.add("sa_full")
    for i in (1, 2, 3):
        if not np.all(inp[f"ln{i}_g"] == 1.0):
            flags.add(f"ln{i}_g")
        if np.any(inp[f"ln{i}_b"] != 0.0):
            flags.add(f"ln{i}_b")
    if np.any(inp["fc_b2"] != 0.0):
        flags.add("b2")
    flags = frozenset(flags)

    if flags not in _CACHE:
        _CACHE[flags] = _build(flags)
    nc = _CACHE[flags]

    shared = _prep_shared(inp)
    if "b2" in flags:
        shared["fc_b2"] = inp["fc_b2"].reshape(1, H).astype(np.float32)
    for i in (1, 2, 3):
        if f"ln{i}_g" in flags:
            shared[f"ln{i}_g"] = inp[f"ln{i}_g"].reshape(1, H).astype(np.float32)
        if f"ln{i}_b" in flags:
            shared[f"ln{i}_b"] = inp[f"ln{i}_b"].reshape(1, H).astype(np.float32)

    in_maps = []
    for c in range(8):
        m = _prep_core(c, x, kv, future_mask, mask, "ca_mask" in flags)
        m.update(shared)
        in_maps.append(m)

    from concourse import bass_utils

    t0 = time.perf_counter_ns()
    res = bass_utils.run_bass_kernel_spmd(
        nc, in_maps, core_ids=list(range(8)), trace=False
    )
    LAST_RUN_NS = time.perf_counter_ns() - t0

    out = np.empty((B, S, H), np.float32)
    for c in range(8):
        b, half = c // 2, c % 2
        out[b, own_rows(half)] = res.results[c]["out"]
    return out



# revision 16
# speedup vs baseline: 20.8496x; 1.0151x over previous
"""Trainium2 Bass kernel for nn_DecoderBlock (B=4, S=1024, H=1024, 16 heads).

Sharding (8 cores, zero cross-core communication):
  core c -> batch b = c//2, half = c%2; own query rows are the interleaved
  256-row blocks {B0,B3} (half 0) / {B1,B2} (half 1) -- see own_rows().
  Row-parallel over the sequence for LN / projections / FFN; each core of a
  batch pair duplicates the full K/V projections (they depend only on the
  INPUTS x / key_and_value, never on the other core's partial results).

Device layout strategy:
  - fp16 operands for every matmul (1 cyc/row on PE), fp32 PSUM accumulate.
  - Attention in transposed-score space: scoresT[k, q] = kT.T @ qT per head,
    exp on ACT (scale=1/8 folded in), causal/padding mask as a 0/1 fp16
    MULTIPLY on the exp output (host-derived from the boolean masks; exact),
    softmax denominator via a ones-column appended to V (one extra PSUM row),
    normalized with reciprocal + gpsimd partition_broadcast.  ctxT feeds the
    output projection directly as lhsT - attention is never transposed.
  - Rows are host-permuted own-first so the uniform SPMD program slices "my
    queries" at column 0; softmax attention is key-order invariant and the
    mask tiles are built in permuted key order.  The interleaved block
    sharding makes the causally-clipped SA_SCHED (75% of the full rectangle,
    alternating full-width / upper-half-only key-chunk pairs) valid for BOTH
    cores of a pair with one uniform program; data masks cover the rest.

Engine balance (cost-model-guided; PE busy ~388us is the floor at fp16):
  - LN rstd = exp(-0.5*ln(var+eps)) on ACT: the whole program then lives on
    the one ACT table set holding {exp, ln, copy, relu}; no 2.7us reloads.
  - LN transpose evictions on ACT (scalar.copy); projection evictions stay
    on DVE -- the two streams overlap in the PE-bound projection phases.
  - v_aug is [v(64)|ones] (65-wide lhsT slice): the old 63 zero pad columns
    fed PSUM rows that were never read; their memset was pure DVE waste.
  - qT zero padding (the head-masking mechanism) is written once at build
    time into a persistent tile; per-iteration memsets dropped.
  - kv LN emitted inside the SA projection window (engines drain their
    streams in emission order) so CA's K/V inputs are ready early.
"""

import sys

sys.path.insert(0, "/opt/trn_rl_repo")

import time
from contextlib import ExitStack

import numpy as np

import concourse.bass as bass
import concourse.mybir as mybir
import concourse.tile as tile
from concourse import bacc
from concourse.masks import make_identity

F32 = mybir.dt.float32
F16 = mybir.dt.float16
AF = mybir.ActivationFunctionType
OP = mybir.AluOpType

B, S, H, NH, DK, FF = 4, 1024, 1024, 16, 64, 4096
P = 128
HT = H // P  # 8 feature tiles of the model dim
QN = 512  # own query rows per core
QC = QN // P  # 4 query chunks
RC = S // P  # 8 key/row chunks
FT = FF // P  # 32 ffn tiles
NEG = -50000.0  # fp16-safe -inf surrogate (exp(NEG/8) == 0 in fp32)
VW = 72  # per-head width of v_aug: [v(64) | ones | 7 pad cols never touched]
EPS = 1e-5

_CACHE: dict = {}
LAST_RUN_NS: int | None = None


def _ilv(k):
    """Row shuffle so that SBUF [P, k, n] loaded with "(p k) n -> p k n"
    (contiguous k*rowbytes per partition) holds orig row 128*c+p at
    (partition p, chunk c):  shuf[k*p + c] = orig[128*c + p]."""
    return (np.arange(k)[None, :] * 128 + np.arange(128)[:, None]).reshape(-1)


IDX8 = _ilv(8)
IDX4 = _ilv(4)
IDX32 = _ilv(32)


def _bcast_row_ap(dram_ap, parts=P):
    """DRAM [1, N] -> partition-broadcast AP [parts, N] (step-0 partition dim)."""
    return bass.AP(
        tensor=dram_ap.tensor, offset=dram_ap.offset, ap=[[0, parts], dram_ap.ap[1]]
    )


def _build(flags: frozenset, repeat: int = 1, ablate: frozenset = frozenset()):
    """Build + compile the single SPMD program. `flags` toggles optional ops."""
    use_ca_mask = "ca_mask" in flags
    use_sa_full = "sa_full" in flags
    ln_g = {i: f"ln{i}_g" in flags for i in (1, 2, 3)}
    ln_b = {i: f"ln{i}_b" in flags for i in (1, 2, 3)}
    use_b2 = "b2" in flags

    nc = bacc.Bacc("TRN2", target_bir_lowering=False, debug=False, num_devices=8)

    D = {}

    def din(name, shape, dt):
        D[name] = nc.dram_tensor(name, shape, dt, kind="ExternalInput").ap()

    din("x_own", [QN, H], F16)
    din("x_rm", [S, H], F16)  # permuted rows (own first)
    din("kv_rm", [S, H], F16)
    din("sa_cb", [P, RC, QN], F16)  # additive causal bias, permuted key order
    if use_ca_mask:
        din("ca_cb", [P, RC, QN], F16)
    for w in ("sa_wq", "sa_wk", "sa_wv", "sa_wo", "ca_wq", "ca_wk", "ca_wv", "ca_wo"):
        din(w, [H, H], F16)
    din("fc_w1", [H, FF], F16)
    din("fc_w2", [FF, H], F16)
    din("fc_b1f", [P, FT], F32)  # feature-major b1
    if use_b2:
        din("fc_b2", [1, H], F32)
    for i in (1, 2, 3):
        if ln_g[i]:
            din(f"ln{i}_g", [1, H], F32)
        if ln_b[i]:
            din(f"ln{i}_b", [1, H], F32)
    out_d = nc.dram_tensor("out", [QN, H], F32, kind="ExternalOutput").ap()

    with tile.TileContext(nc) as tc, ExitStack() as top:
        sing = top.enter_context(tc.tile_pool(name="sing", bufs=1))
        ident = sing.tile([P, P], F16)
        make_identity(nc, ident)
        eps_t = sing.tile([P, 1], F32)
        nc.vector.memset(eps_t, EPS)
        ones_c = sing.tile([P, 1], F32)
        nc.vector.memset(ones_c, 1.0)

        # broadcast [1,H] params across partitions via step-0 DMA
        gb_tiles = {}
        for i in (1, 2, 3):
            for kind, on in (("g", ln_g[i]), ("b", ln_b[i])):
                if on:
                    t = sing.tile([P, H], F32)
                    nc.sync.dma_start(out=t, in_=_bcast_row_ap(D[f"ln{i}_{kind}"]))
                    gb_tiles[(i, kind)] = t
        b1f = sing.tile([P, FT], F32)
        nc.sync.dma_start(out=b1f, in_=D["fc_b1f"])
        b2_t = None
        if use_b2:
            b2_t = sing.tile([P, H], F32)
            nc.sync.dma_start(out=b2_t, in_=_bcast_row_ap(D["fc_b2"]))

        small = top.enter_context(tc.tile_pool(name="small", bufs=4))
        norms = top.enter_context(tc.tile_pool(name="norms", bufs=2))
        scratch = top.enter_context(tc.tile_pool(name="scratch", bufs=2))
        lnzp = top.enter_context(tc.tile_pool(name="lnzp", bufs=1))

        # persistent q tile: the zero halves (the head-masking mechanism for
        # the padded-contraction score matmuls) are written once here and
        # never touched again; SA and CA overwrite only their data rows.
        qTp = sing.tile([P, NH, QN], F16, tag="qTp")
        nc.vector.memset(qTp, 0.0)

        def emit_once():
         xown16 = sing.tile([P, QC, H], F16, tag="xown")
         nc.sync.dma_start(
             out=xown16, in_=D["x_own"].rearrange("(p q) n -> p q n", p=P)
         )
         sacb = sing.tile([P, RC, QN], F16, tag="sacb")
         nc.sync.dma_start(out=sacb, in_=D["sa_cb"])
         cacb = None
         if use_ca_mask:
             cacb = sing.tile([P, RC, QN], F16, tag="cacb")
             nc.sync.dma_start(out=cacb, in_=D["ca_cb"])
         z = sing.tile([P, QC, H], F16, tag="z")
         z2 = sing.tile([P, QC, H], F16, tag="z2")

         mm_stack = ExitStack()
         ps_mm = mm_stack.enter_context(
             tc.tile_pool(name="ps_mm", bufs=2, space="PSUM")
         )

         def layernorm_T(row_ap_fn, n_rc, lnT, ln_idx, ps_t, tag):
            """Row-major LN stats+apply, then PE-transpose into feature-major lnT.

            row_ap_fn(rc) -> SBUF AP [P, H] holding rows rc*128..+128.
            lnT: [P, HT, n_rc*P] fp16 tile (feat-in-tile, feat-tile, row).
            """
            for rc in range(n_rc):
                row = row_ap_fn(rc)
                st = small.tile([P, 2, 6], F32, tag="st")
                nc.vector.bn_stats(out=st[:, 0], in_=row[:, 0:512])
                nc.vector.bn_stats(out=st[:, 1], in_=row[:, 512:H])
                mv = small.tile([P, 2], F32, tag="mv")
                nc.vector.bn_aggr(out=mv, in_=st)
                # rstd = exp(-0.5*ln(var+eps)): keeps ACT on the one table set
                # that also serves Exp/Copy/Relu (no 2.7us table reloads), and
                # frees DVE of the reciprocal.
                lnv = small.tile([P, 1], F32, tag="lnv")
                nc.scalar.activation(
                    out=lnv, in_=mv[:, 1:2], func=AF.Ln, bias=eps_t, scale=1.0
                )
                rstd = small.tile([P, 1], F32, tag="rstd")
                nc.scalar.activation(out=rstd, in_=lnv, func=AF.Exp, scale=-0.5)
                lnr = scratch.tile([P, H], F16, tag="lnr")
                nc.vector.tensor_scalar(
                    out=lnr,
                    in0=row,
                    scalar1=mv[:, 0:1],
                    scalar2=rstd,
                    op0=OP.subtract,
                    op1=OP.mult,
                )
                if (ln_idx, "g") in gb_tiles:
                    nc.vector.tensor_mul(out=lnr, in0=lnr, in1=gb_tiles[(ln_idx, "g")])
                if (ln_idx, "b") in gb_tiles:
                    nc.vector.tensor_add(out=lnr, in0=lnr, in1=gb_tiles[(ln_idx, "b")])
                for f in range(HT):
                    pt = ps_t.tile([P, P], F16, tag="pt")
                    nc.tensor.transpose(pt, lnr[:, f * P : (f + 1) * P], ident)
                    # evict on ACT: DVE is busy with LN stats/apply in these
                    # phases while ACT is idle
                    nc.scalar.copy(
                        out=lnT[:, f, rc * P : (rc + 1) * P], in_=pt
                    )

         _w8_cache = {}

         def load_w8(dram, pool):
            if "nowdma" in ablate:
                if "w8" not in _w8_cache:
                    t = pool.tile([P, HT, H], F16, tag="w8")
                    nc.sync.dma_start(
                        out=t, in_=dram.rearrange("(p k) n -> p k n", p=P)
                    )
                    _w8_cache["w8"] = t
                return _w8_cache["w8"]
            t = pool.tile([P, HT, H], F16, tag="w8")
            nc.sync.dma_start(out=t, in_=dram.rearrange("(p k) n -> p k n", p=P))
            return t

         FULL_SCHED = ((512, 0),) * RC
         # uniform causally-clipped SA schedule (see _prep_core block layout):
         # kc pairs alternate full-width and upper-half-only (q cols 256..511)
         SA_SCHED = (
             (512, 0), (512, 0), (256, 256), (256, 256),
             (512, 0), (512, 0), (256, 256), (256, 256),
         )

         def attention(qT, kT, v_aug, ctxT, cb, ps_s, ps_av, exp_pool, sched):
            for h in range(NH):
                f, r0 = h // 2, (h % 2) * 64
                pm_av = (
                    None
                    if "noav" in ablate
                    else ps_av.tile([P, QN], F32, tag="av")
                )
                av_out = None if pm_av is None else pm_av[0:65]
                for g in range(4):
                    n, off = sched[2 * g]  # both kc of a pair share (n, off)
                    et = exp_pool.tile([P, 2, QN], F16, tag="et")
                    if "noscores" in ablate:
                        nc.vector.memset(et, 0.01)
                    else:
                        pm_s = ps_s.tile([P, 2, QN], F32, tag="s")
                        for j in range(2):
                            kc = g * 2 + j
                            # K=128 contraction: other head's rows in qT are 0
                            nc.tensor.matmul(
                                pm_s[:, j, 0:n],
                                lhsT=kT[:, f, kc * P : (kc + 1) * P],
                                rhs=qT[:, h, off : off + n],
                                start=True,
                                stop=True,
                            )
                        if "noexp" in ablate:
                            nc.vector.tensor_copy(
                                out=et[:, :, 0:n], in_=pm_s[:, :, 0:n]
                            )
                        else:
                            nc.scalar.activation(
                                out=et[:, :, 0:n],
                                in_=pm_s[:, :, 0:n],
                                func=AF.Exp,
                                scale=0.125,
                            )
                        if cb is not None and "nobias" not in ablate:
                            # exp(s/8)*m == masked softmax numerator (m in {0,1})
                            nc.vector.tensor_mul(
                                out=et[:, :, 0:n],
                                in0=et[:, :, 0:n],
                                in1=cb[:, 2 * g : 2 * g + 2, off : off + n],
                            )
                    if "noav" not in ablate:
                        for j in range(2):
                            kc = g * 2 + j
                            # lhsT = [v(64) | ones]: psum row 64 accumulates
                            # sumexp; only psum rows 0:65 are written/read.
                            nc.tensor.matmul(
                                av_out[:, off : off + n],
                                lhsT=v_aug[:, kc, h, 0:65],
                                rhs=et[:, j, 0:n],
                                start=(kc == 0),
                                stop=(kc == RC - 1),
                            )
                if "noav" in ablate:
                    nc.vector.memset(ctxT[r0 : r0 + 64, f, :], 0.25)
                elif "nonorm" in ablate:
                    nc.vector.tensor_copy(
                        out=ctxT[r0 : r0 + 64, f, :], in_=pm_av[0:64, :]
                    )
                else:
                    recip = norms.tile([1, QN], F16, tag="rec")
                    with nc.allow_low_precision(
                        reason="1/sumexp in fp16 is within output tolerance"
                    ):
                        nc.vector.reciprocal(out=recip, in_=pm_av[64:65, :])
                    rb = norms.tile([64, QN], F16, tag="rb")
                    nc.gpsimd.partition_broadcast(out_ap=rb, in_ap=recip)
                    nc.vector.tensor_mul(
                        out=ctxT[r0 : r0 + 64, f, :], in0=pm_av[0:64, :], in1=rb
                    )

         def proj_heads_qpad(qT_pad, w_sb, lnT):
            # qT_pad[:, h, :]: head h q-dims at rows (h%2)*64..+64 (matching its
            # row range inside the packed kT tile f=h//2), other 64 rows zero
            # (zeroed once at build time -- qT_pad is the persistent qTp).
            for f in range(HT):
                pm = ps_mm.tile([P, 512], F32, tag="proj")
                for kc in range(HT):
                    nc.tensor.matmul(
                        pm,
                        lhsT=w_sb[:, kc, f * P : (f + 1) * P],
                        rhs=lnT[:, kc, 0:QN],
                        start=(kc == 0),
                        stop=(kc == HT - 1),
                    )
                nc.vector.tensor_copy(out=qT_pad[0:64, 2 * f, :], in_=pm[0:64, :])
                nc.vector.tensor_copy(
                    out=qT_pad[64:128, 2 * f + 1, :], in_=pm[64:128, :]
                )

         def proj_to_featmajor(outT, w_sb, lnT, n_cols):
            # outT[:, f, c*512:+512] = sum_kc w[kc,f]^T @ lnT[kc, cols]
            for f in range(HT):
                for c in range(n_cols // 512):
                    pm = ps_mm.tile([P, 512], F32, tag="proj")
                    for kc in range(HT):
                        nc.tensor.matmul(
                            pm,
                            lhsT=w_sb[:, kc, f * P : (f + 1) * P],
                            rhs=lnT[:, kc, c * 512 : (c + 1) * 512],
                            start=(kc == 0),
                            stop=(kc == HT - 1),
                        )
                    nc.vector.tensor_copy(
                        out=outT[:, f, c * 512 : (c + 1) * 512], in_=pm
                    )

         def make_v_aug(v_aug, w_sb, lnT):
            # cols 65:VW are never written nor read (AV lhsT slices 0:65)
            nc.vector.tensor_copy(
                out=v_aug[:, :, :, 64:65], in_=ones_c.to_broadcast([P, RC, NH, 1])
            )
            for kc in range(RC):
                for vc in range(2):
                    pm = ps_mm.tile([P, 512], F32, tag="proj")
                    for hc in range(HT):
                        nc.tensor.matmul(
                            pm,
                            lhsT=lnT[:, hc, kc * P : (kc + 1) * P],
                            rhs=w_sb[:, hc, vc * 512 : (vc + 1) * 512],
                            start=(hc == 0),
                            stop=(hc == HT - 1),
                        )
                    nc.vector.tensor_copy(
                        out=v_aug[:, kc, vc * 8 : (vc + 1) * 8, 0:64],
                        in_=pm.rearrange("p (h d) -> p h d", h=8),
                    )

         def wo_residual(ctxT, w_sb, base, out_rows):
            # out_rows[:, qc, :] = base[:, qc, :] + ctx @ wo
            for qc in range(QC):
                for ncol in range(2):
                    pm = ps_mm.tile([P, 512], F32, tag="proj")
                    for hd in range(HT):
                        nc.tensor.matmul(
                            pm,
                            lhsT=ctxT[:, hd, qc * P : (qc + 1) * P],
                            rhs=w_sb[:, hd, ncol * 512 : (ncol + 1) * 512],
                            start=(hd == 0),
                            stop=(hd == HT - 1),
                        )
                    sl = slice(ncol * 512, (ncol + 1) * 512)
                    nc.vector.tensor_tensor(
                        out=out_rows[:, qc, sl], in0=pm, in1=base[:, qc, sl], op=OP.add
                    )

         lnz = lnzp.tile([P, HT, QN], F16, tag="lnzT")

         with tc.tile_pool(name="attn_acts", bufs=1) as acts, tc.tile_pool(
            name="wpool", bufs=2
         ) as wpool:
            # ---------------- P0: LN1(x) -> ln1T ----------------
            # quarter the DMA so LN of the first chunks starts early
            x_sb = acts.tile([P, RC, H], F16, tag="kT")
            x_ap = D["x_rm"].rearrange("(p k) n -> p k n", p=P)
            for i in range(4):
                nc.sync.dma_start(
                    out=x_sb[:, 2 * i : 2 * i + 2, :], in_=x_ap[:, 2 * i : 2 * i + 2, :]
                )
            ln1T = acts.tile([P, HT, S], F16, tag="lnT")
            kv_sb = acts.tile([P, RC, H], F16, tag="kvrows")
            kv_ap = D["kv_rm"].rearrange("(p k) n -> p k n", p=P)
            nc.sync.dma_start(out=kv_sb[:, 0:4, :], in_=kv_ap[:, 0:4, :])
            nc.sync.dma_start(out=kv_sb[:, 4:8, :], in_=kv_ap[:, 4:8, :])
            ln2kvT = acts.tile([P, HT, S], F16, tag="lnT2")
            with tc.tile_pool(name="ps_t1", bufs=2, space="PSUM") as ps_t:
                layernorm_T(lambda rc: x_sb[:, rc, :], RC, ln1T, 1, ps_t, "l1")

                # ------------- P1: SA projections -------------
                # kv LN is emitted between the projections: its DVE/ACT work
                # fills the PE-bound projection window (engines execute their
                # streams in emission order), keeping it off the critical path
                # well before CA needs it.
                qT = qTp
                kT = acts.tile([P, HT, S], F16, tag="kT")
                v_aug = acts.tile([P, RC, NH, VW], F16, tag="vaug")
                wq = load_w8(D["sa_wq"], wpool)
                proj_heads_qpad(qT, wq, ln1T)
                wk = load_w8(D["sa_wk"], wpool)
                proj_to_featmajor(kT, wk, ln1T, S)
                layernorm_T(lambda rc: kv_sb[:, rc, :], RC, ln2kvT, 2, ps_t, "l2kv")
                wv = load_w8(D["sa_wv"], wpool)
                make_v_aug(v_aug, wv, ln1T)

            # ---------------- SA attention ----------------
            ctxT = acts.tile([P, HT, QN], F16, tag="ctxT")
            if "noattn" in ablate:
                nc.vector.memset(ctxT, 0.25)
            else:
                with (
                    tc.tile_pool(name="ps_s1", bufs=2, space="PSUM") as ps_s,
                    tc.tile_pool(name="ps_av1", bufs=2, space="PSUM") as ps_av,
                    tc.tile_pool(name="exp1", bufs=3) as exp_pool,
                ):
                    attention(
                     qT, kT, v_aug, ctxT, sacb, ps_s, ps_av, exp_pool,
                     FULL_SCHED if use_sa_full else SA_SCHED,
                 )

            # ---------------- SA wo + residual -> z ----------------
            wo = load_w8(D["sa_wo"], wpool)
            wo_residual(ctxT, wo, xown16, z)

            # ---------------- P2: cross attention ----------------
            # CA K/V projections depend only on ln2kvT (ready since P0) and
            # can fill PE while SA attention is ACT(exp)-bound; kT2/v_aug2
            # reuse SA slots so scheduling overlap is limited to what WAR
            # hazards allow.
            cwk = load_w8(D["ca_wk"], wpool)
            kT2 = acts.tile([P, HT, S], F16, tag="kT")
            proj_to_featmajor(kT2, cwk, ln2kvT, S)
            cwv = load_w8(D["ca_wv"], wpool)
            v_aug2 = acts.tile([P, RC, NH, VW], F16, tag="vaug")
            make_v_aug(v_aug2, cwv, ln2kvT)

            with tc.tile_pool(name="ps_t2", bufs=2, space="PSUM") as ps_t:
                layernorm_T(lambda rc: z[:, rc, :], QC, lnz, 2, ps_t, "l2z")
            qT2 = qTp
            cwq = load_w8(D["ca_wq"], wpool)
            proj_heads_qpad(qT2, cwq, lnz)

            ctxT2 = acts.tile([P, HT, QN], F16, tag="ctxT")
            if "noattn" in ablate:
                nc.vector.memset(ctxT2, 0.25)
            else:
                with (
                    tc.tile_pool(name="ps_s2", bufs=2, space="PSUM") as ps_s,
                    tc.tile_pool(name="ps_av2", bufs=2, space="PSUM") as ps_av,
                    tc.tile_pool(name="exp2", bufs=3) as exp_pool,
                ):
                    attention(qT2, kT2, v_aug2, ctxT2, cacb, ps_s, ps_av, exp_pool, FULL_SCHED)

            cwo = load_w8(D["ca_wo"], wpool)
            wo_residual(ctxT2, cwo, z, z2)

         # ---------------- P3: FFN ----------------
         with tc.tile_pool(name="ps_t3", bufs=2, space="PSUM") as ps_t:
            layernorm_T(lambda rc: z2[:, rc, :], QC, lnz, 3, ps_t, "l3")

         with (
             tc.tile_pool(name="hTpool", bufs=1) as hTpool,
             tc.tile_pool(name="w2pool", bufs=3) as w2pool,
         ):
            hT = hTpool.tile([P, FT, QN], F16, tag="hT")
            w2_ap = D["fc_w2"].rearrange("(p k) n -> p k n", p=P)
            with tc.tile_pool(name="w1pool", bufs=1) as w1pool:
                # two halves so hT matmuls start after the first 4MB lands
                w1a = w1pool.tile([P, 4, FF], F16, tag="w1a")
                nc.sync.dma_start(
                    out=w1a,
                    in_=D["fc_w1"].rearrange("(p k) n -> p k n", p=P)[:, 0:4, :],
                )
                w1b = w1pool.tile([P, 4, FF], F16, tag="w1b")
                nc.sync.dma_start(
                    out=w1b,
                    in_=D["fc_w1"].rearrange("(p k) n -> p k n", p=P)[:, 4:8, :],
                )
                for ft in range(FT):
                    pm = ps_mm.tile([P, 512], F32, tag="proj")
                    for kc in range(HT):
                        w1t = w1a if kc < 4 else w1b
                        nc.tensor.matmul(
                            pm,
                            lhsT=w1t[:, kc % 4, ft * P : (ft + 1) * P],
                            rhs=lnz[:, kc, :],
                            start=(kc == 0),
                            stop=(kc == HT - 1),
                        )
                    nc.scalar.activation(
                        out=hT[:, ft, :],
                        in_=pm,
                        func=AF.Relu,
                        bias=b1f[:, ft : ft + 1],
                    )

            mm_stack.close()  # free ps_mm banks for ps_big
            out_rows = sing.tile([P, QC, H], F32, tag="z")  # reuses z slot
            with tc.tile_pool(name="ps_big", bufs=1, space="PSUM") as ps_big:
                pm8 = ps_big.tile([P, 8, 512], F32)
                w2t = None
                for kc in range(FT):
                    if kc % 4 == 0:
                        w2t = w2pool.tile([P, 4, H], F16, tag="w2s")
                        nc.sync.dma_start(
                            out=w2t, in_=w2_ap[:, kc : kc + 4, :]
                        )
                    for qc in range(QC):
                        for ncol in range(2):
                            nc.tensor.matmul(
                                pm8[:, qc * 2 + ncol, :],
                                lhsT=hT[:, kc, qc * P : (qc + 1) * P],
                                rhs=w2t[:, kc % 4, ncol * 512 : (ncol + 1) * 512],
                                start=(kc == 0),
                                stop=(kc == FT - 1),
                            )
                out_ap = out_d.rearrange("(q p) n -> p q n", p=P)
                for qc in range(QC):
                    for ncol in range(2):
                        sl = slice(ncol * 512, (ncol + 1) * 512)
                        nc.vector.tensor_tensor(
                            out=out_rows[:, qc, sl],
                            in0=pm8[:, qc * 2 + ncol, :],
                            in1=z2[:, qc, sl],
                            op=OP.add,
                        )
                        if b2_t is not None:
                            nc.vector.tensor_add(
                                out=out_rows[:, qc, sl],
                                in0=out_rows[:, qc, sl],
                                in1=b2_t[:, sl],
                            )
                    # stream each query chunk out as soon as it is complete
                    nc.sync.dma_start(
                        out=out_ap[:, qc : qc + 1, :],
                        in_=out_rows[:, qc : qc + 1, :],
                    )

        for _ in range(repeat):
            emit_once()

    nc.compile()
    return nc


def own_rows(half):
    """Query rows of a core: blocks {B0,B3} / {B1,B2} of 256 rows each.
    This interleaving is what makes the uniform SA_SCHED causally valid
    for both cores of a batch pair."""
    if half == 0:
        return np.concatenate([np.arange(0, 256), np.arange(768, 1024)])
    return np.arange(256, 768)


def _prep_core(c, x, kv, future_mask, mask, use_ca_mask):
    b, half = c // 2, c % 2
    own = own_rows(half)
    if half == 0:
        rest = np.concatenate([np.arange(256, 512), np.arange(512, 768)])
    else:
        rest = np.concatenate([np.arange(0, 256), np.arange(768, 1024)])
    perm = np.concatenate([own, rest])
    m = {}
    m["x_own"] = np.ascontiguousarray(x[b, own][IDX4]).astype(np.float16)
    m["x_rm"] = np.ascontiguousarray(x[b][perm][IDX8]).astype(np.float16)
    m["kv_rm"] = np.ascontiguousarray(kv[b][IDX8]).astype(np.float16)
    # sa_cb[p, kc, q] = 0 where future_mask[b, own_q, perm_key] else 1 (key=kc*128+p)
    fm = future_mask[b, own][:, perm]  # [QN, S] bool
    cb = np.where(fm.T, np.float16(0.0), np.float16(1.0))  # [S, QN]
    m["sa_cb"] = np.ascontiguousarray(cb.reshape(RC, P, QN).transpose(1, 0, 2))
    if use_ca_mask:
        cm = mask[b, own]  # [QN, S]
        ccb = np.where(cm.T, np.float16(0.0), np.float16(1.0))
        m["ca_cb"] = np.ascontiguousarray(ccb.reshape(RC, P, QN).transpose(1, 0, 2))
    return m


def _prep_shared(inp):
    shared = {}
    for w in ("sa_wq", "sa_wk", "sa_wv", "sa_wo", "ca_wq", "ca_wk", "ca_wv", "ca_wo"):
        shared[w] = np.ascontiguousarray(np.asarray(inp[w])[IDX8]).astype(np.float16)
    shared["fc_w1"] = np.ascontiguousarray(
        np.asarray(inp["fc_w1"])[IDX8]
    ).astype(np.float16)
    shared["fc_w2"] = np.ascontiguousarray(
        np.asarray(inp["fc_w2"])[IDX32]
    ).astype(np.float16)
    shared["fc_b1f"] = np.ascontiguousarray(
        np.asarray(inp["fc_b1"]).reshape(FT, P).T
    ).astype(np.float32)
    return shared


def kernel(**inputs) -> np.ndarray:
    global LAST_RUN_NS
    inp = {k: np.asarray(v) for k, v in inputs.items()}
    x, kv = inp["x"], inp["key_and_value"]
    mask, future_mask = inp["mask"], inp["future_mask"]

    flags = set()
    if mask.any():
        flags.add("ca_mask")
    # The clipped SA_SCHED structurally skips regions that a standard causal
    # mask guarantees are masked.  Only safe if future_mask IS causal triu;
    # otherwise fall back to the full-rectangle schedule (mask data covers it).
    tri = np.triu(np.ones((S, S), dtype=bool), 1)
    if not all(np.array_equal(future_mask[b], tri) for b in range(B)):
        flags.add("sa_full")
    for i in (1, 2, 3):
        if not np.all(inp[f"ln{i}_g"] == 1.0):
            flags.add(f"ln{i}_g")
        if np.any(inp[f"ln{i}_b"] != 0.0):
            flags.add(f"ln{i}_b")
    if np.any(inp["fc_b2"] != 0.0):
        flags.add("b2")
    flags = frozenset(flags)

    if flags not in _CACHE:
        _CACHE[flags] = _build(flags)
    nc = _CACHE[flags]

    shared = _prep_shared(inp)
    if "b2" in flags:
        shared["fc_b2"] = inp["fc_b2"].reshape(1, H).astype(np.float32)
    for i in (1, 2, 3):
        if f"ln{i}_g" in flags:
            shared[f"ln{i}_g"] = inp[f"ln{i}_g"].reshape(1, H).astype(np.float32)
        if f"ln{i}_b" in flags:
            shared[f"ln{i}_b"] = inp[f"ln{i}_b"].reshape(1, H).astype(np.float32)

    in_maps = []
    for c in range(8):
        m = _prep_core(c, x, kv, future_mask, mask, "ca_mask" in flags)
        m.update(shared)
        in_maps.append(m)

    from concourse import bass_utils

    t0 = time.perf_counter_ns()
    res = bass_utils.run_bass_kernel_spmd(
        nc, in_maps, core_ids=list(range(8)), trace=False
    )
    LAST_RUN_NS = time.perf_counter_ns() - t0

    out = np.empty((B, S, H), np.float32)
    for c in range(8):
        b, half = c // 2, c % 2
        out[b, own_rows(half)] = res.results[c]["out"]
    return out

